# revision 100
# baseline (speedup 1.0000x reference)
"""Trainium2 Bass kernel for nn_Block_with_lora (dense transformer block).

Sharding: 8 cores = 4 batches x 2 token-parity shards (stride-2 over T).
Each core computes its 512 query tokens end-to-end (no collectives);
K/V projections over all 1024 tokens are computed per-core.

v2: LoRA folded into weights host-side (W+scale*A*B); K-bias dropped
(softmax shift-invariant over keys); V-bias folded into the following
projection's bias; multiplicative post-exp mask (GpSimd) instead of
additive band; single x load with in-place bf16 LN; single-pass MLP
with fc/pr interleave; V2/K2 projections emitted as in-order PE filler
inside the attention phases so the tensor engine never idles (TRN2
p-state ramps to 2.4GHz only after ~3us of continuous PE activity).
"""

import sys

sys.path.insert(0, "/opt/trn_rl_repo")

import numpy as np
import ml_dtypes
from collections import deque
from contextlib import ExitStack

BF = ml_dtypes.bfloat16

C = 1024
H = 16
DH = 64
R = 16
SCALE = 1.0 / R
T = 1024
TQ = 512
NT = 8  # C / 128
EPS = 1e-5
NCORES = 8

_PROG = None


def _build_program(debug=False):
    import concourse.bass as bass
    import concourse.tile as tile
    from concourse import mybir, bacc

    f32 = mybir.dt.float32
    bf16 = mybir.dt.bfloat16
    AF = mybir.ActivationFunctionType
    AL = mybir.AluOpType

    nc = bacc.Bacc("TRN2", target_bir_lowering=False, debug=False)

    def din(name, shape, dt=f32):
        return nc.dram_tensor(name, shape, dt, kind="ExternalInput").ap()

    xT_d = din("xT", [C, T])
    xqT_d = din("xqT", [C, TQ])
    fT_d = din("fT", [C, T])
    mask_d = din("mask", [128, 64], bf16)
    sel_d = din("sel", [4, 8, 128], mybir.dt.float32r)

    w_d = {}
    for n in ["wq", "wk", "wv", "wsp", "wcq", "wck", "wcv", "wcp"]:
        w_d[n] = din(n, [C, C], bf16)
    w_d["wfc"] = din("wfc", [C, 4 * C], bf16)
    w_d["wpr"] = din("wpr", [4 * C, C], bf16)
    bias_d = {
        n: din(n, [C], f32)
        for n in ["bq", "bsp", "bcq", "bcp", "bpr", "g1", "b1", "g2", "b2"]
    }
    bias_d["bfc"] = din("bfc", [4 * C], f32)

    outT_d = nc.dram_tensor("outT", [C, TQ], f32, kind="ExternalOutput").ap()
    dbg = {}
    if debug:
        for n, sh, dt in [("dbg_lnb", [C, T], bf16), ("dbg_lnown", [C, TQ], bf16),
                          ("dbg_q", [C, TQ], bf16), ("dbg_k", [C, T], bf16),
                          ("dbg_o", [C, TQ], bf16), ("dbg_r1", [C, TQ], f32),
                          ("dbg_o2", [C, TQ], bf16), ("dbg_r2", [C, TQ], f32),
                          ("dbg_k2", [C, T], bf16)]:
            dbg[n] = nc.dram_tensor(n, sh, dt, kind="ExternalOutput").ap()

    with tile.TileContext(nc) as tc, ExitStack() as ctx:

        def pool(name, bufs, space=None):
            kw = dict(name=name, bufs=bufs)
            if space:
                kw["space"] = space
            return ctx.enter_context(tc.tile_pool(**kw))

        # ---- SBUF pools ----
        acts = pool("acts", 16)      # [128,1024] bf16: xb->lnb (8) + fb (8)  32KB
        kpool = pool("kpool", 8)     # [128,1024] bf16: kT (later m_sb 0..7)  16KB
        k2pool = pool("k2pool", 8)   # [128,1024] bf16: k2T (later m_sb 8..15) 16KB
        vpool = pool("vpool", 16)    # [128,1040] bf16: vt+v2t (later m_sb 16..31) 32.5KB
        qpool = pool("qpool", 8)     # [128,512] bf16: qT -> q2T               8KB
        lnsm = pool("lnsm", 8)       # [128,512] bf16: own_b->lnown/ln1b/ln2   8KB
        opool = pool("opool", 8)     # [128,512] bf16: oT -> o2T               8KB
        rpool = pool("rpool", 8)     # [128,512] f32: residual (persist)      16KB
        wpool = pool("wpool", 14)    # [128,512] bf16 weight chunks           14KB
        vwpool = pool("vwpool", 8)   # [128,512] bf16 wcv chunks (filler)      8KB
        epool = pool("epool", 2)     # [128,1024] bf16: exp(S)                 4KB
        big32 = pool("big32", 2)     # [128,1024] f32: x/f stream              8KB
        sbig = pool("sbig", 2)       # [128,1024] bf16: mb/rb bcast            4KB
        sqpool = pool("sqpool", 2)   # [128,1024] bf16: squares                4KB
        rows = pool("rows", 2)       # [1,1024] f32 stat rows                  8KB
        rrows = pool("rrows", 1)     # [1,512] f32 softmax denom rows          2KB
        recb = pool("recb", 1)       # [128,512] bf16 recip bcast              1KB
        dallp = pool("dallp", 1)     # [16,512] f32 batched denoms             2KB
        outfp = pool("outfp", 1)     # [128,512] f32 out staging               2KB
        smalls = pool("smalls", 1)   # bias/g/b columns per tag
        onesp = pool("onesp", 1)
        maskp = pool("maskp", 1)

        # ---- PSUM pools: 2*2 + 4*1 = 8 banks ----
        pA = pool("pA", 2, space="PSUM")   # [128,1024] f32 (2 banks each)
        pB = pool("pB", 4, space="PSUM")   # [128,512] f32 (1 bank each)

        # ---- constants ----
        ones_c16 = onesp.tile([128, 1], bf16, tag="oc16")
        nc.gpsimd.memset(ones_c16[:], 1.0)
        ones_r32 = onesp.tile([1, 128], f32, tag="or32")
        nc.gpsimd.memset(ones_r32[:], 1.0)
        eps_t = onesp.tile([1, 1], f32, tag="eps")
        nc.gpsimd.memset(eps_t[:], EPS)

        # weight DMA round-robin over (gpsimd, sync) queues
        dma_rr = [0]

        def wdma(dst, src):
            eng = (nc.gpsimd, nc.sync)[dma_rr[0] % 2]
            dma_rr[0] += 1
            eng.dma_start(dst, src)

        # =============== input streams ===============
        # x first (it heads the longest serial chain: stats -> LN -> k/v
        # projections), then xq, then f (emitted later, after the qkv
        # weight DMAs, so queues deliver in need-order)
        xb = []
        for k in range(NT):
            xt = big32.tile([128, T], f32, tag="big32", name=f"xstream{k}")
            (nc.sync if k % 2 == 1 else nc.gpsimd).dma_start(
                xt[:], xT_d[k * 128:(k + 1) * 128, :])
            xbt = acts.tile([128, T], bf16, tag="acts", name=f"xb{k}")
            nc.vector.tensor_copy(xbt[:], xt[:])
            xb.append(xbt)
        resid = []
        for k in range(NT):
            rt = rpool.tile([128, TQ], f32, tag="rpool", name=f"resid{k}")
            (nc.sync if k % 2 == 0 else nc.gpsimd).dma_start(
                rt[:], xqT_d[k * 128:(k + 1) * 128, :])
            resid.append(rt)
        fb = []  # filled by load_f() below, after qkv weight DMAs
        # ones columns for V denominators
        vt = [vpool.tile([128, 1040], bf16, tag="vpool", name=f"vt{i}") for i in range(NT)]
        v2t = [vpool.tile([128, 1040], bf16, tag="vpool", name=f"v2t{i}") for i in range(NT)]
        for tt in range(NT):
            nc.gpsimd.memset(vt[tt][:, 64:1040:65], 1.0)
            nc.gpsimd.memset(v2t[tt][:, 64:1040:65], 1.0)

        # =============== helpers ===============
        def bcast_rows_f32(row_f32, out_sb, Tn):
            # broadcast [1,Tn] f32 row to [128,Tn] f32 SBUF via K=1 PE matmul
            for hh in range(Tn // 512):
                sl = slice(hh * 512, (hh + 1) * 512)
                bp = pB.tile([128, 512], f32, tag="pB", name="bcp")
                nc.tensor.matmul(bp[:], ones_r32[0:1, :], row_f32[0:1, sl],
                                 start=True, stop=True)
                nc.vector.tensor_copy(out_sb[:, sl], bp[:])

        def stat_rows(mean_ps, sq_ps, Tn):
            """mean/rstd rows from accumulated sum / sumsq psums (Tn<=512)."""
            mean_row = rows.tile([1, Tn], f32, tag="rows", name="meanr")
            rstd_row = rows.tile([1, Tn], f32, tag="rows", name="rstdr")
            nc.vector.tensor_scalar_mul(mean_row[:], mean_ps[:], 1.0 / C)
            nc.vector.tensor_mul(rstd_row[:], mean_row[:], mean_row[:])
            nc.vector.scalar_tensor_tensor(rstd_row[:], sq_ps[:], 1.0 / C, rstd_row[:],
                                           op0=AL.mult, op1=AL.subtract)
            nc.scalar.activation(rstd_row[:], rstd_row[:], AF.Sqrt, bias=eps_t[:])
            nc.vector.reciprocal(rstd_row[:], rstd_row[:])
            return mean_row, rstd_row

        def ln_small(src_f32_tiles, g_col, b_col, out_tiles, emit_filler=None):
            """LN over channel (partition) dim for [128,TQ] f32 tiles.
            f32 intermediates throughout (score precision is exp-amplified)."""
            mean_ps = pA.tile([1, TQ], f32, tag="pA", name="mps")
            sq_ps = pA.tile([1, TQ], f32, tag="pA", name="sps")
            for k in range(NT):
                nc.vector.tensor_copy(out_tiles[k][:], src_f32_tiles[k][:])
                sq = sqpool.tile([128, TQ], bf16, tag="sqf", name="sqo")
                nc.scalar.activation(sq[:], out_tiles[k][:], AF.Square)
                nc.tensor.matmul(mean_ps[:], ones_c16[:], out_tiles[k][:],
                                 start=(k == 0), stop=(k == NT - 1))
                nc.tensor.matmul(sq_ps[:], ones_c16[:], sq[:],
                                 start=(k == 0), stop=(k == NT - 1))
            if emit_filler is not None:
                emit_filler(1)
            mean_row, rstd_row = stat_rows(mean_ps, sq_ps, TQ)
            mb = sbig.tile([128, TQ], f32, tag="sbig", name="mbs")
            rb = sbig.tile([128, TQ], f32, tag="sbig", name="rbs")
            bcast_rows_f32(mean_row, mb, TQ)
            bcast_rows_f32(rstd_row, rb, TQ)
            if emit_filler is not None:
                emit_filler(1)
            for k in range(NT):
                t32 = big32.tile([128, TQ], f32, tag="big32", name="lnt32")
                nc.vector.tensor_sub(t32[:], src_f32_tiles[k][:], mb[:])
                nc.vector.tensor_mul(t32[:], t32[:], rb[:])
                nc.scalar.activation(out_tiles[k][:], t32[:], AF.Identity,
                                     bias=b_col[:, k:k + 1], scale=g_col[:, k:k + 1])

        def projT(wname, rhs_tiles, Tn, drain, mh_range=(0, 1), interleave=None,
                  wqueue=None):
            """outT[mi] psum groups; drain(mi, h, pt) consumes each."""
            for mh in mh_range:
                wts = []
                for k in range(NT):
                    wt = wpool.tile([128, 512], bf16, tag="wpool", name="wt")
                    if wqueue is None:
                        wdma(wt[:], w_d[wname][k * 128:(k + 1) * 128,
                                               mh * 512:(mh + 1) * 512])
                    else:
                        wqueue.dma_start(wt[:], w_d[wname][k * 128:(k + 1) * 128,
                                                           mh * 512:(mh + 1) * 512])
                    wts.append(wt)
                for ml in range(4):
                    mi = mh * 4 + ml
                    for h in range(Tn // 512):
                        pt = pB.tile([128, 512], f32, tag="pB", name="pt")
                        for k in range(NT):
                            nc.tensor.matmul(pt[:], wts[k][:, ml * 128:(ml + 1) * 128],
                                             rhs_tiles[k][:, h * 512:(h + 1) * 512],
                                             start=(k == 0), stop=(k == NT - 1))
                        drain(mi, pt, h)
                        if interleave is not None:
                            interleave(mh, ml, h)

        def v_drain(v_tiles, tt, dh, pt):
            dest = v_tiles[tt][:, dh * 520:(dh + 1) * 520]
            dest = dest.rearrange("p (h d) -> p h d", d=65)[:, :, 0:64]
            nc.vector.tensor_copy(dest, pt[:])

        def proj_V(wname, lhs_tiles, v_tiles, dh_range=(0, 1)):
            for dh in dh_range:
                wts = []
                for k in range(NT):
                    wt = wpool.tile([128, 512], bf16, tag="wpool", name="vwt")
                    wdma(wt[:], w_d[wname][k * 128:(k + 1) * 128,
                                           dh * 512:(dh + 1) * 512])
                    wts.append(wt)
                for tt in range(NT):
                    pt = pB.tile([128, 512], f32, tag="pB", name="vpt")
                    for k in range(NT):
                        nc.tensor.matmul(pt[:], lhs_tiles[k][:, tt * 128:(tt + 1) * 128],
                                         wts[k][:], start=(k == 0), stop=(k == NT - 1))
                    v_drain(v_tiles, tt, dh, pt)

        # ---------------- filler machinery ----------------
        # Each filler item is a closure emitting ~4 PE matmuls. Groups are
        # split into two chunks (A: k0-3 start, B: k4-7 stop + drain) that
        # must be emitted within the same attention head (pB slot safety).
        fillers = deque()

        def emit_filler(n):
            for _ in range(min(n, len(fillers))):
                fillers.popleft()()

        def drain_fillers():
            while fillers:
                fillers.popleft()()

        def make_v2_fillers():
            # full psum groups (8 matmuls + drain) as self-contained fillers;
            # wcv weight chunks in their own pool, scalar queue (independent
            # of the x/f streams)
            for dh in range(2):
                wts = []
                for k in range(NT):
                    wt = vwpool.tile([128, 512], bf16, tag="vw", name="vw")
                    nc.scalar.dma_start(wt[:], w_d["wcv"][k * 128:(k + 1) * 128,
                                                          dh * 512:(dh + 1) * 512])
                    wts.append(wt)
                for tt in range(NT):
                    def grp(dh=dh, tt=tt, wts=wts):
                        pt = pB.tile([128, 512], f32, tag="pB", name="v2pt")
                        for k in range(NT):
                            nc.tensor.matmul(pt[:], fb[k][:, tt * 128:(tt + 1) * 128],
                                             wts[k][:], start=(k == 0),
                                             stop=(k == NT - 1))
                        v_drain(v2t, tt, dh, pt)
                    fillers.append(grp)

        def make_k2_fillers(mh):
            wts = []
            for k in range(NT):
                wt = vwpool.tile([128, 512], bf16, tag="vw", name="k2w")
                nc.scalar.dma_start(wt[:], w_d["wck"][k * 128:(k + 1) * 128,
                                                      mh * 512:(mh + 1) * 512])
                wts.append(wt)
            for ml in range(4):
                mi = 4 * mh + ml
                for h in range(2):
                    def grp(mi=mi, ml=ml, h=h, wts=wts):
                        pt = pB.tile([128, 512], f32, tag="pB", name="k2pt")
                        for k in range(NT):
                            nc.tensor.matmul(pt[:], wts[k][:, ml * 128:(ml + 1) * 128],
                                             fb[k][:, h * 512:(h + 1) * 512],
                                             start=(k == 0), stop=(k == NT - 1))
                        nc.vector.tensor_copy(k2T[mi][:, h * 512:(h + 1) * 512],
                                              pt[:])
                    fillers.append(grp)

        # ---------------- attention ----------------
        f32r = mybir.dt.float32r

        def attention(q_tiles, k_tiles, v_tiles, o_tiles, pre_pair=None):
            dq = [None]

            def rescale_start():
                # fast recip of 8 denom rows, rounded to f32r for the PE
                dquad = dq[0]
                nc.vector.reciprocal_approx_fast(dquad[:], dquad[:])
                dr = dallp.tile([8, 512], f32r, tag="dallr", name="dallr", bufs=2)
                nc.vector.tensor_copy(dr[:], dquad[:])
                return dr

            def rescale_pair(dr, j, mi2):
                bp = pB.tile([128, 512], f32, tag="pB", name="selbp")
                nc.tensor.matmul(bp[:], sel_t[j][:], dr[:], start=True, stop=True)
                rbc = recb.tile([128, 512], bf16, tag="recb", name="rbc")
                nc.vector.tensor_copy(rbc[:], bp[:])
                nc.vector.tensor_mul(o_tiles[mi2][:], o_tiles[mi2][:], rbc[:])

            dr0 = [None]
            for h in range(H):
                mi, off = h // 2, 64 * (h % 2)
                if h % 8 == 0:
                    dq[0] = dallp.tile([8, 512], f32, tag="dall", name="dquad",
                                       bufs=2)
                if pre_pair is not None and h % 2 == 0:
                    pre_pair(mi)
                    emit_filler(1)
                if 8 <= h <= 11:
                    # quad-0 pair rescales deferred here so their recip
                    # chain never stalls the PE
                    rescale_pair(dr0[0], h - 8, h - 8)
                op = pB.tile([65, 512], f32, tag="pB", name="op")
                ets = [None] * 4

                def score(jp):
                    kj0, kj1 = 2 * jp, 2 * jp + 1
                    q0, q1 = 64 * kj0, 64 * kj1
                    e1 = 512 + (512 - q1)
                    st = pA.tile([128, 1024], f32, tag="pA", name="st")
                    nc.tensor.matmul(
                        st[:, q0:512],
                        k_tiles[mi][off:off + 64, kj0 * 128:(kj0 + 1) * 128],
                        q_tiles[mi][off:off + 64, q0:512],
                        start=True, stop=True)
                    nc.tensor.matmul(
                        st[:, 512:e1],
                        k_tiles[mi][off:off + 64, kj1 * 128:(kj1 + 1) * 128],
                        q_tiles[mi][off:off + 64, q1:512],
                        start=True, stop=True)
                    et = epool.tile([128, 1024], bf16, tag="epool", name="et")
                    nc.scalar.activation(et[:, q0:e1], st[:, q0:e1], AF.Exp)
                    # multiplicative causal mask on diagonal-straddling blocks
                    nc.vector.tensor_mul(et[:, q0:q0 + 64], et[:, q0:q0 + 64], mask_t[:])
                    nc.vector.tensor_mul(et[:, 512:576], et[:, 512:576], mask_t[:])
                    ets[jp] = et

                def av(jp):
                    kj0, kj1 = 2 * jp, 2 * jp + 1
                    q0, q1 = 64 * kj0, 64 * kj1
                    e1 = 512 + (512 - q1)
                    et = ets[jp]
                    nc.tensor.matmul(
                        op[:] if kj0 == 0 else op[:, q0:512],
                        v_tiles[kj0][:, 65 * h:65 * h + 65],
                        et[:, q0:512], start=(kj0 == 0), stop=False)
                    nc.tensor.matmul(
                        op[:, q1:512],
                        v_tiles[kj1][:, 65 * h:65 * h + 65],
                        et[:, 512:e1], start=False, stop=(kj1 == 7))

                score(0)
                for jp in range(4):
                    if jp == 1 and h >= 4:
                        # skip early heads (PE still busy on ramp) so the
                        # leftover groups drain into the attention tail
                        emit_filler(1)
                    if jp < 3:
                        score(jp + 1)
                    av(jp)
                # stash raw head output + denominator row
                nc.vector.tensor_copy(o_tiles[mi][off:off + 64, :], op[0:64, :])
                rr = rrows.tile([1, 512], f32, tag="rr", name="rr")
                nc.vector.tensor_copy(rr[:], op[64:65, :])
                nc.gpsimd.dma_start(dq[0][h % 8:h % 8 + 1, :], rr[:])
                if h == 7:
                    dr0[0] = rescale_start()
                elif h == 15:
                    dr1 = rescale_start()
                    for j in range(4):
                        rescale_pair(dr1, j, 4 + j)

        # =============== phase 1: k2 (mh0) + full-x LN stats ===============
        k2T = [k2pool.tile([128, T], bf16, tag="k2", name=f"k2T{i}") for i in range(NT)]
        xsq = []
        for k in range(NT):
            sq = sqpool.tile([128, T], bf16, tag="sqf", name=f"xsq{k}")
            # Square on the (idle) Scalar engine keeps DVE free for casts
            nc.scalar.activation(sq[:], xb[k][:], AF.Square)
            xsq.append(sq)

        # small constants on the scalar queue, emitted AFTER the Square work
        # so their sequencer dispatch cost never delays the LN stats; the
        # tiles themselves are first read ~25us in (lnb scale/bias)
        mask_t = maskp.tile([128, 64], bf16, tag="mask")
        nc.scalar.dma_start(mask_t[:], mask_d[:, :])
        sel_t = []
        for j in range(4):
            st_ = smalls.tile([8, 128], mybir.dt.float32r, tag=f"sel{j}",
                              name=f"sel{j}")
            nc.scalar.dma_start(st_[:], sel_d[j])
            sel_t.append(st_)

        def load_percol(name, n=NT):
            t = smalls.tile([128, n], f32, tag=name, name=name)
            nc.scalar.dma_start(t[:], bias_d[name].rearrange("(m p) -> p m", p=128))
            return t

        bias_t = {
            n: load_percol(n)
            for n in ["bq", "bsp", "bcq", "bcp", "bpr", "g1", "b1", "g2", "b2"]
        }
        bias_t["bfc"] = load_percol("bfc", 32)

        mean_ps = pA.tile([1, T], f32, tag="pA", name="meanps")
        sq_ps = pA.tile([1, T], f32, tag="pA", name="sqps")
        stat_cnt = [0]

        def emit_stats_upto(n):
            while stat_cnt[0] < n:
                k = stat_cnt[0]
                for hh in range(2):
                    sl = slice(hh * 512, (hh + 1) * 512)
                    nc.tensor.matmul(mean_ps[0:1, sl], ones_c16[:], xb[k][:, sl],
                                     start=(k == 0), stop=(k == NT - 1))
                    nc.tensor.matmul(sq_ps[0:1, sl], ones_c16[:], xsq[k][:, sl],
                                     start=(k == 0), stop=(k == NT - 1))
                stat_cnt[0] += 1

        emit_stats_upto(NT)

        # full-x LN: rows + bcast per 512-half, f32 intermediates
        mb_f = sbig.tile([128, T], f32, tag="sbig", name="mbf")
        rb_f = sbig.tile([128, T], f32, tag="sbig", name="rbf")
        for hh in range(2):
            sl = slice(hh * 512, (hh + 1) * 512)
            mean_row, rstd_row = stat_rows(mean_ps[0:1, sl], sq_ps[0:1, sl], 512)
            bcast_rows_f32(mean_row, mb_f[:, sl], 512)
            bcast_rows_f32(rstd_row, rb_f[:, sl], 512)

        lnb = xb  # bf16 x tiles overwritten with LN output
        for k in range(NT):
            t32 = big32.tile([128, T], f32, tag="big32", name="lnt32f")
            nc.vector.tensor_sub(t32[:], xb[k][:], mb_f[:])
            nc.vector.tensor_mul(t32[:], t32[:], rb_f[:])
            nc.scalar.activation(lnb[k][:], t32[:], AF.Identity,
                                 bias=bias_t["b1"][:, k:k + 1],
                                 scale=bias_t["g1"][:, k:k + 1])

        def dump(name, tiles):
            if debug:
                for k in range(NT):
                    nc.sync.dma_start(dbg[name][k * 128:(k + 1) * 128, :],
                                      tiles[k][:])

        dump("dbg_lnb", lnb)

        # =============== phase 2: k, v projections ===============
        kT = [kpool.tile([128, T], bf16, tag="kT", name=f"kT{i}") for i in range(NT)]

        def k_drain(mi, pt, h):
            if (mi + h) % 2 == 0:
                nc.vector.tensor_copy(kT[mi][:, h * 512:(h + 1) * 512], pt[:])
            else:
                nc.scalar.copy(kT[mi][:, h * 512:(h + 1) * 512], pt[:])

        projT("wk", lnb, T, k_drain)
        dump("dbg_k", kT)
        proj_V("wv", lnb, vt)

        # own-token LN from residual (f32) -> lnown; emitted after wk/wv so
        # its slow input chain (xq stream + busy DVE) never blocks them in
        # PE program order (lnown is first needed by wq inside attention)
        lnown = [lnsm.tile([128, TQ], bf16, tag="lnsm", name=f"lnown{i}")
                 for i in range(NT)]
        ln_small(resid, bias_t["g1"], bias_t["b1"], lnown)
        dump("dbg_lnown", lnown)

        # q-projection groups are emitted inside the attention head loop
        # (pre_pair) so they fill the Scalar-exp stalls
        def make_q_pre(wname, src_tiles, out_tiles, bias_name):
            wts_cur = {}

            def load_half(mh):
                wts = []
                for k in range(NT):
                    wt = wpool.tile([128, 512], bf16, tag="wpool", name="qw")
                    wdma(wt[:], w_d[wname][k * 128:(k + 1) * 128,
                                           mh * 512:(mh + 1) * 512])
                    wts.append(wt)
                wts_cur[mh] = wts

            def pre(mi):
                if mi == 0:
                    load_half(0)
                elif mi == 2:
                    load_half(1)  # prefetch ahead of pair 4
                ml = mi % 4
                wts = wts_cur[mi // 4]
                pt = pB.tile([128, 512], f32, tag="pB", name="qpt")
                for k in range(NT):
                    nc.tensor.matmul(pt[:], wts[k][:, ml * 128:(ml + 1) * 128],
                                     src_tiles[k][:], start=(k == 0),
                                     stop=(k == NT - 1))
                nc.scalar.activation(out_tiles[mi][:], pt[:], AF.Identity,
                                     bias=bias_t[bias_name][:, mi:mi + 1])
            return pre

        # f stream now (queues deliver after the qkv weights), then the
        # feature-side filler groups become available
        for k in range(NT):
            ft = big32.tile([128, T], f32, tag="big32", name=f"fstream{k}")
            (nc.sync if k % 2 == 0 else nc.gpsimd).dma_start(
                ft[:], fT_d[k * 128:(k + 1) * 128, :])
            fbt = acts.tile([128, T], bf16, tag="acts", name=f"fb{k}")
            nc.vector.tensor_copy(fbt[:], ft[:])
            fb.append(fbt)
        make_k2_fillers(0)
        make_v2_fillers()

        # =============== phase 3: self attention (wq + K2/V2 fillers) =======
        qT = [qpool.tile([128, TQ], bf16, tag="q", name=f"qT{i}") for i in range(NT)]
        oT = [opool.tile([128, TQ], bf16, tag="o", name=f"oT{i}") for i in range(NT)]
        attention(qT, kT, vt, oT,
                  pre_pair=make_q_pre("wq", lnown, qT, "bq"))
        drain_fillers()
        dump("dbg_q", qT)
        dump("dbg_o", oT)

        # =============== phase 4: self proj + residual ===============
        def sp_drain(mi, pt, h):
            nc.vector.scalar_tensor_tensor(resid[mi][:], pt[:],
                                           bias_t["bsp"][:, mi:mi + 1],
                                           resid[mi][:], op0=AL.add, op1=AL.add)

        projT("wsp", oT, TQ, sp_drain)
        dump("dbg_r1", resid)

        # =============== phase 5: LN1 on updated own tokens ===============
        make_k2_fillers(1)
        ln1b = [lnsm.tile([128, TQ], bf16, tag="lnsm", name=f"ln1b{i}")
                for i in range(NT)]
        ln_small(resid, bias_t["g1"], bias_t["b1"], ln1b, emit_filler=emit_filler)

        # =============== phase 6+7: cross attention (wcq + k2 fillers) ======
        q2T = [qpool.tile([128, TQ], bf16, tag="q", name=f"q2T{i}") for i in range(NT)]
        o2T = [opool.tile([128, TQ], bf16, tag="o", name=f"o2T{i}") for i in range(NT)]
        attention(q2T, k2T, v2t, o2T,
                  pre_pair=make_q_pre("wcq", ln1b, q2T, "bcq"))
        drain_fillers()
        dump("dbg_o2", o2T)
        dump("dbg_k2", k2T)

        # =============== phase 8: cross proj + residual ===============
        def cp_drain(mi, pt, h):
            nc.vector.scalar_tensor_tensor(resid[mi][:], pt[:],
                                           bias_t["bcp"][:, mi:mi + 1],
                                           resid[mi][:], op0=AL.add, op1=AL.add)

        projT("wcp", o2T, TQ, cp_drain)
        dump("dbg_r2", resid)

        # =============== phase 9: LN2 + MLP ===============
        def load_fc_w(grp):
            wts = []
            for k in range(NT):
                wt = wpool.tile([128, 512], bf16, tag="wpool", name="fcw")
                wdma(wt[:], w_d["wfc"][k * 128:(k + 1) * 128,
                                       grp * 512:(grp + 1) * 512])
                wts.append(wt)
            return wts

        fc_w0 = load_fc_w(0)  # prefetch while LN2 runs
        ln2 = [lnsm.tile([128, TQ], bf16, tag="lnsm", name=f"ln2_{i}")
               for i in range(NT)]
        ln_small(resid, bias_t["g2"], bias_t["b2"], ln2)

        # hidden tiles reuse dead kT/k2T/v slots (no extra SBUF)
        m_sb = []
        for i in range(32):
            if i < 8:
                mt = kpool.tile([128, TQ], bf16, tag="kT", name=f"m{i}")
            elif i < 16:
                mt = k2pool.tile([128, TQ], bf16, tag="k2", name=f"m{i}")
            else:
                mt = vpool.tile([128, TQ], bf16, tag="vpool", name=f"m{i}")
            m_sb.append(mt)

        # pr accumulators quad0 (mi 0..3) live across fc; fc psums from pA
        pr_ps0 = [pB.tile([128, TQ], f32, tag="pB", name=f"pr0_{j}")
                  for j in range(4)]

        def fc_group(grp, wts):
            for ml in range(4):
                mi = grp * 4 + ml
                pt = pA.tile([128, TQ], f32, tag="pA", name="fcpt")
                for k in range(NT):
                    nc.tensor.matmul(pt[:, 0:TQ], wts[k][:, ml * 128:(ml + 1) * 128],
                                     ln2[k][:], start=(k == 0), stop=(k == NT - 1))
                nc.scalar.activation(m_sb[mi][:], pt[:, 0:TQ], AF.Gelu_apprx_tanh,
                                     bias=bias_t["bfc"][:, mi:mi + 1])

        def pr_q0_group(k):
            wt = wpool.tile([128, 512], bf16, tag="wpool", name="prw")
            wdma(wt[:], w_d["wpr"][k * 128:(k + 1) * 128, 0:512])
            for j in range(4):
                nc.tensor.matmul(pr_ps0[j][:], wt[:, j * 128:(j + 1) * 128],
                                 m_sb[k][:], start=(k == 0), stop=(k == 31))

        # fc groups with lagged pr-quad0 interleave (pr group k after fc
        # group covering hidden tile k is complete)
        pr_done = [0]

        def pump_pr(n):
            while pr_done[0] < n:
                pr_q0_group(pr_done[0])
                pr_done[0] += 1

        fc_wts = {0: fc_w0}
        for grp in range(8):
            wts = fc_wts.pop(grp)
            if grp + 1 < 8:
                fc_wts[grp + 1] = load_fc_w(grp + 1)
            fc_group(grp, wts)
            if grp >= 1:
                pump_pr(4 * grp)   # lag one group behind gelu
        pump_pr(32)

        def emit_out(quad, qts):
            for j in range(4):
                mi = quad * 4 + j
                of = outfp.tile([128, TQ], f32, tag="outf", name="of")
                nc.vector.scalar_tensor_tensor(of[:], qts[j][:],
                                               bias_t["bpr"][:, mi:mi + 1],
                                               resid[mi][:], op0=AL.add, op1=AL.add)
                nc.sync.dma_start(outT_d[mi * 128:(mi + 1) * 128, :], of[:])

        emit_out(0, pr_ps0)

        # pr quad1 (mi 4..7): straight accumulation, all m_sb ready
        pr_ps1 = [pB.tile([128, TQ], f32, tag="pB", name=f"pr1_{j}")
                  for j in range(4)]
        for k in range(32):
            wt = wpool.tile([128, 512], bf16, tag="wpool", name="prw1")
            wdma(wt[:], w_d["wpr"][k * 128:(k + 1) * 128, 512:1024])
            for j in range(4):
                nc.tensor.matmul(pr_ps1[j][:], wt[:, j * 128:(j + 1) * 128],
                                 m_sb[k][:], start=(k == 0), stop=(k == 31))
        emit_out(1, pr_ps1)

    nc.compile()
    return nc


def _get_program():
    global _PROG
    if _PROG is None:
        _PROG = _build_program()
    return _PROG


def _prep_shared(inputs):
    g = {}

    def bf(a):
        return np.ascontiguousarray(np.asarray(a, dtype=np.float32)).astype(BF)

    def f(a):
        return np.ascontiguousarray(np.asarray(a, dtype=np.float32))

    def fold(w, a, lb):
        # effective W^T (in->out layout) with LoRA folded:
        # y = x W^T + (x A^T) B^T s  ->  W_eff^T = W^T + A^T B^T s
        return np.asarray(w, np.float32).T + \
            np.asarray(a, np.float32).T @ np.asarray(lb, np.float32).T * SCALE

    qw, kw, vw = (inputs["sa_qkv_w"][i * C:(i + 1) * C] for i in range(3))
    qb, kb, vb = (inputs["sa_qkv_b"][i * C:(i + 1) * C] for i in range(3))
    qlb, klb, vlb = (inputs["sa_qkv_lb"][i * C:(i + 1) * C] for i in range(3))
    inv = 1.0 / np.sqrt(DH)
    a_sa = inputs["sa_qkv_a"]
    g["wq"] = bf(fold(qw, a_sa, qlb) * inv)
    g["wk"] = bf(fold(kw, a_sa, klb))
    g["wv"] = bf(fold(vw, a_sa, vlb))
    g["bq"] = f(np.asarray(qb) * inv)
    # K bias dropped: adds a per-query constant to all logits (softmax
    # shift-invariant over keys). V bias folded into the next projection.
    g["wsp"] = bf(fold(inputs["sa_proj_w"], inputs["sa_proj_a"], inputs["sa_proj_lb"]))
    g["bsp"] = f(np.asarray(inputs["sa_proj_b"], np.float32) +
                 np.asarray(inputs["sa_proj_w"], np.float32) @ np.asarray(vb, np.float32))

    g["wcq"] = bf(fold(inputs["ca_q_w"], inputs["ca_q_a"], inputs["ca_q_lb"]) * inv)
    g["bcq"] = f(np.asarray(inputs["ca_q_b"]) * inv)

    ckw, cvw = inputs["ca_kv_w"][0:C], inputs["ca_kv_w"][C:2 * C]
    cvb = inputs["ca_kv_b"][C:2 * C]
    cklb, cvlb = inputs["ca_kv_lb"][0:C], inputs["ca_kv_lb"][C:2 * C]
    a_ck = inputs["ca_kv_a"]
    g["wck"] = bf(fold(ckw, a_ck, cklb))
    g["wcv"] = bf(fold(cvw, a_ck, cvlb))

    g["wcp"] = bf(fold(inputs["ca_proj_w"], inputs["ca_proj_a"], inputs["ca_proj_lb"]))
    g["bcp"] = f(np.asarray(inputs["ca_proj_b"], np.float32) +
                 np.asarray(inputs["ca_proj_w"], np.float32) @ np.asarray(cvb, np.float32))

    g["wfc"] = bf(np.asarray(inputs["fc_w"]).T)
    g["bfc"] = f(inputs["fc_b"])
    g["wpr"] = bf(np.asarray(inputs["pr_w"]).T)
    g["bpr"] = f(inputs["pr_b"])
    g["g1"] = f(inputs["ln1_g"])
    g["b1"] = f(inputs["ln1_b"])
    g["g2"] = f(inputs["ln2_g"])
    g["b2"] = f(inputs["ln2_b"])

    sel = np.zeros((4, 8, 128), np.float32)
    for j in range(4):
        sel[j, 2 * j, 0:64] = 1.0
        sel[j, 2 * j + 1, 64:128] = 1.0
    g["sel"] = sel
    return g


def _make_in_maps(inputs):
    inputs = {k: np.asarray(v) for k, v in inputs.items()}
    x, feat = inputs["x"], inputs["feature"]
    B = x.shape[0]
    shared = _prep_shared(inputs)

    masks = []
    for p in range(2):
        jj = np.arange(128).reshape(128, 1)
        ii = np.arange(64).reshape(1, 64)
        live = jj <= 2 * ii + p
        masks.append(np.where(live, 1.0, 0.0).astype(np.float32).astype(BF))

    in_maps = []
    xTs = [np.ascontiguousarray(np.asarray(x[b]).T, dtype=np.float32) for b in range(B)]
    fTs = [np.ascontiguousarray(np.asarray(feat[b]).T, dtype=np.float32) for b in range(B)]
    for core in range(NCORES):
        b, p = core // 2, core % 2
        m = dict(shared)
        m["xT"] = xTs[b]
        m["xqT"] = np.ascontiguousarray(xTs[b][:, p::2])
        m["fT"] = fTs[b]
        m["mask"] = masks[p]
        in_maps.append(m)
    return in_maps, B


def kernel(**inputs):
    from concourse.bass_utils import run_bass_kernel_spmd

    nc = _get_program()
    in_maps, B = _make_in_maps(inputs)
    res = run_bass_kernel_spmd(nc, in_maps, core_ids=list(range(NCORES)))
    out = np.zeros((B, T, C), np.float32)
    for core in range(NCORES):
        b, p = core // 2, core % 2
        out[b, p::2, :] = np.asarray(res.results[core]["outT"], dtype=np.float32).T
    return out


# revision 107
# speedup vs baseline: 1.0187x; 1.0187x over previous
"""Trainium2 Bass kernel for nn_Block_with_lora (dense transformer block).

Sharding: 8 cores = 4 batches x 2 token-parity shards (stride-2 over T).
Each core computes its 512 query tokens end-to-end (no collectives);
K/V projections over all 1024 tokens are computed per-core.

v2: LoRA folded into weights host-side (W+scale*A*B); K-bias dropped
(softmax shift-invariant over keys); V-bias folded into the following
projection's bias; multiplicative post-exp mask (GpSimd) instead of
additive band; single x load with in-place bf16 LN; single-pass MLP
with fc/pr interleave; V2/K2 projections emitted as in-order PE filler
inside the attention phases so the tensor engine never idles (TRN2
p-state ramps to 2.4GHz only after ~3us of continuous PE activity).
"""

import sys

sys.path.insert(0, "/opt/trn_rl_repo")

import numpy as np
import ml_dtypes
from collections import deque
from contextlib import ExitStack

BF = ml_dtypes.bfloat16

C = 1024
H = 16
DH = 64
R = 16
SCALE = 1.0 / R
T = 1024
TQ = 512
NT = 8  # C / 128
EPS = 1e-5
NCORES = 8

_PROG = None


def _build_program(debug=False):
    import concourse.bass as bass
    import concourse.tile as tile
    from concourse import mybir, bacc

    f32 = mybir.dt.float32
    bf16 = mybir.dt.bfloat16
    AF = mybir.ActivationFunctionType
    AL = mybir.AluOpType

    nc = bacc.Bacc("TRN2", target_bir_lowering=False, debug=False)

    def din(name, shape, dt=f32):
        return nc.dram_tensor(name, shape, dt, kind="ExternalInput").ap()

    xT_d = din("xT", [C, T])
    xqT_d = din("xqT", [C, TQ])
    fT_d = din("fT", [C, T])
    mask_d = din("mask", [128, 64], bf16)
    sel_d = din("sel", [4, 8, 128], mybir.dt.float32r)

    w_d = {}
    for n in ["wq", "wk", "wv", "wsp", "wcq", "wck", "wcv", "wcp"]:
        w_d[n] = din(n, [C, C], bf16)
    w_d["wfc"] = din("wfc", [C, 4 * C], bf16)
    w_d["wpr"] = din("wpr", [4 * C, C], bf16)
    bias_d = {
        n: din(n, [C], f32)
        for n in ["bq", "bsp", "bcq", "bcp", "bpr", "g1", "b1", "g2", "b2"]
    }
    bias_d["bfc"] = din("bfc", [4 * C], f32)

    outT_d = nc.dram_tensor("outT", [C, TQ], f32, kind="ExternalOutput").ap()
    dbg = {}
    if debug:
        for n, sh, dt in [("dbg_lnb", [C, T], bf16), ("dbg_lnown", [C, TQ], bf16),
                          ("dbg_q", [C, TQ], bf16), ("dbg_k", [C, T], bf16),
                          ("dbg_o", [C, TQ], bf16), ("dbg_r1", [C, TQ], f32),
                          ("dbg_o2", [C, TQ], bf16), ("dbg_r2", [C, TQ], f32),
                          ("dbg_k2", [C, T], bf16)]:
            dbg[n] = nc.dram_tensor(n, sh, dt, kind="ExternalOutput").ap()

    with tile.TileContext(nc) as tc, ExitStack() as ctx:

        def pool(name, bufs, space=None):
            kw = dict(name=name, bufs=bufs)
            if space:
                kw["space"] = space
            return ctx.enter_context(tc.tile_pool(**kw))

        # ---- SBUF pools ----
        acts = pool("acts", 16)      # [128,1024] bf16: xb->lnb (8) + fb (8)  32KB
        kpool = pool("kpool", 8)     # [128,1024] bf16: kT (later m_sb 0..7)  16KB
        k2pool = pool("k2pool", 8)   # [128,1024] bf16: k2T (later m_sb 8..15) 16KB
        vpool = pool("vpool", 16)    # [128,1040] bf16: vt+v2t (later m_sb 16..31) 32.5KB
        qpool = pool("qpool", 8)     # [128,512] bf16: qT -> q2T               8KB
        lnsm = pool("lnsm", 8)       # [128,512] bf16: own_b->lnown/ln1b/ln2   8KB
        opool = pool("opool", 8)     # [128,512] bf16: oT -> o2T               8KB
        rpool = pool("rpool", 8)     # [128,512] f32: residual (persist)      16KB
        wpool = pool("wpool", 14)    # [128,512] bf16 weight chunks           14KB
        vwpool = pool("vwpool", 8)   # [128,512] bf16 wcv chunks (filler)      8KB
        epool = pool("epool", 2)     # [128,1024] bf16: exp(S)                 4KB
        big32 = pool("big32", 2)     # [128,1024] f32: x/f stream              8KB
        sbig = pool("sbig", 2)       # [128,1024] bf16: mb/rb bcast            4KB
        sqpool = pool("sqpool", 2)   # [128,1024] bf16: squares                4KB
        rows = pool("rows", 2)       # [1,1024] f32 stat rows                  8KB
        rrows = pool("rrows", 1)     # [1,512] f32 softmax denom rows          2KB
        recb = pool("recb", 1)       # [128,512] bf16 recip bcast              1KB
        dallp = pool("dallp", 1)     # [16,512] f32 batched denoms             2KB
        outfp = pool("outfp", 1)     # [128,512] f32 out staging               2KB
        smalls = pool("smalls", 1)   # bias/g/b columns per tag
        onesp = pool("onesp", 1)
        maskp = pool("maskp", 1)

        # ---- PSUM pools: 2*2 + 4*1 = 8 banks ----
        pA = pool("pA", 2, space="PSUM")   # [128,1024] f32 (2 banks each)
        pB = pool("pB", 4, space="PSUM")   # [128,512] f32 (1 bank each)

        # ---- constants ----
        ones_c16 = onesp.tile([128, 1], bf16, tag="oc16")
        nc.gpsimd.memset(ones_c16[:], 1.0)
        ones_r32 = onesp.tile([1, 128], f32, tag="or32")
        nc.gpsimd.memset(ones_r32[:], 1.0)
        eps_t = onesp.tile([1, 1], f32, tag="eps")
        nc.gpsimd.memset(eps_t[:], EPS)

        # weight DMA round-robin over (gpsimd, sync) queues
        dma_rr = [0]

        def wdma(dst, src):
            eng = (nc.gpsimd, nc.sync)[dma_rr[0] % 2]
            dma_rr[0] += 1
            eng.dma_start(dst, src)

        # =============== input streams ===============
        # x first (it heads the longest serial chain: stats -> LN -> k/v
        # projections), then xq, then f (emitted later, after the qkv
        # weight DMAs, so queues deliver in need-order)
        xb = []
        for k in range(NT):
            xt = big32.tile([128, T], f32, tag="big32", name=f"xstream{k}")
            (nc.sync if k % 2 == 1 else nc.gpsimd).dma_start(
                xt[:], xT_d[k * 128:(k + 1) * 128, :])
            xbt = acts.tile([128, T], bf16, tag="acts", name=f"xb{k}")
            nc.vector.tensor_copy(xbt[:], xt[:])
            xb.append(xbt)
        resid = []
        for k in range(NT):
            rt = rpool.tile([128, TQ], f32, tag="rpool", name=f"resid{k}")
            (nc.sync if k % 2 == 0 else nc.gpsimd).dma_start(
                rt[:], xqT_d[k * 128:(k + 1) * 128, :])
            resid.append(rt)
        fb = []  # filled by load_f() below, after qkv weight DMAs
        # ones columns for V denominators
        vt = [vpool.tile([128, 1040], bf16, tag="vpool", name=f"vt{i}") for i in range(NT)]
        v2t = [vpool.tile([128, 1040], bf16, tag="vpool", name=f"v2t{i}") for i in range(NT)]
        for tt in range(NT):
            nc.gpsimd.memset(vt[tt][:, 64:1040:65], 1.0)
            nc.gpsimd.memset(v2t[tt][:, 64:1040:65], 1.0)

        # =============== helpers ===============
        def bcast_rows_f32(row_f32, out_sb, Tn):
            # broadcast [1,Tn] f32 row to [128,Tn] f32 SBUF via K=1 PE matmul
            for hh in range(Tn // 512):
                sl = slice(hh * 512, (hh + 1) * 512)
                bp = pB.tile([128, 512], f32, tag="pB", name="bcp")
                nc.tensor.matmul(bp[:], ones_r32[0:1, :], row_f32[0:1, sl],
                                 start=True, stop=True)
                nc.vector.tensor_copy(out_sb[:, sl], bp[:])

        def stat_rows(mean_ps, sq_ps, Tn):
            """mean/rstd rows from accumulated sum / sumsq psums (Tn<=512)."""
            mean_row = rows.tile([1, Tn], f32, tag="rows", name="meanr")
            rstd_row = rows.tile([1, Tn], f32, tag="rows", name="rstdr")
            nc.vector.tensor_scalar_mul(mean_row[:], mean_ps[:], 1.0 / C)
            nc.vector.tensor_mul(rstd_row[:], mean_row[:], mean_row[:])
            nc.vector.scalar_tensor_tensor(rstd_row[:], sq_ps[:], 1.0 / C, rstd_row[:],
                                           op0=AL.mult, op1=AL.subtract)
            nc.scalar.activation(rstd_row[:], rstd_row[:], AF.Sqrt, bias=eps_t[:])
            # approx recip (~18 bits): the exact DVE reciprocal is a fixed
            # ~3.5us op sitting in every LN boundary chain
            nc.vector.reciprocal_approx_fast(rstd_row[:], rstd_row[:])
            return mean_row, rstd_row

        def ln_small(src_f32_tiles, g_col, b_col, out_tiles, emit_filler=None):
            """LN over channel (partition) dim for [128,TQ] f32 tiles.
            f32 intermediates throughout (score precision is exp-amplified)."""
            mean_ps = pA.tile([1, TQ], f32, tag="pA", name="mps")
            sq_ps = pA.tile([1, TQ], f32, tag="pA", name="sps")
            for k in range(NT):
                nc.vector.tensor_copy(out_tiles[k][:], src_f32_tiles[k][:])
                sq = sqpool.tile([128, TQ], bf16, tag="sqf", name="sqo")
                nc.scalar.activation(sq[:], out_tiles[k][:], AF.Square)
                nc.tensor.matmul(mean_ps[:], ones_c16[:], out_tiles[k][:],
                                 start=(k == 0), stop=(k == NT - 1))
                nc.tensor.matmul(sq_ps[:], ones_c16[:], sq[:],
                                 start=(k == 0), stop=(k == NT - 1))
            if emit_filler is not None:
                emit_filler(1)
            mean_row, rstd_row = stat_rows(mean_ps, sq_ps, TQ)
            mb = sbig.tile([128, TQ], f32, tag="sbig", name="mbs")
            rb = sbig.tile([128, TQ], f32, tag="sbig", name="rbs")
            bcast_rows_f32(mean_row, mb, TQ)
            bcast_rows_f32(rstd_row, rb, TQ)
            if emit_filler is not None:
                emit_filler(1)
            for k in range(NT):
                t32 = big32.tile([128, TQ], f32, tag="big32", name="lnt32")
                nc.vector.tensor_sub(t32[:], src_f32_tiles[k][:], mb[:])
                nc.vector.tensor_mul(t32[:], t32[:], rb[:])
                nc.scalar.activation(out_tiles[k][:], t32[:], AF.Identity,
                                     bias=b_col[:, k:k + 1], scale=g_col[:, k:k + 1])

        def projT(wname, rhs_tiles, Tn, drain, mh_range=(0, 1), interleave=None,
                  wqueue=None):
            """outT[mi] psum groups; drain(mi, h, pt) consumes each."""
            for mh in mh_range:
                wts = []
                for k in range(NT):
                    wt = wpool.tile([128, 512], bf16, tag="wpool", name="wt")
                    if wqueue is None:
                        wdma(wt[:], w_d[wname][k * 128:(k + 1) * 128,
                                               mh * 512:(mh + 1) * 512])
                    else:
                        wqueue.dma_start(wt[:], w_d[wname][k * 128:(k + 1) * 128,
                                                           mh * 512:(mh + 1) * 512])
                    wts.append(wt)
                for ml in range(4):
                    mi = mh * 4 + ml
                    for h in range(Tn // 512):
                        pt = pB.tile([128, 512], f32, tag="pB", name="pt")
                        for k in range(NT):
                            nc.tensor.matmul(pt[:], wts[k][:, ml * 128:(ml + 1) * 128],
                                             rhs_tiles[k][:, h * 512:(h + 1) * 512],
                                             start=(k == 0), stop=(k == NT - 1))
                        drain(mi, pt, h)
                        if interleave is not None:
                            interleave(mh, ml, h)

        def v_drain(v_tiles, tt, dh, pt):
            dest = v_tiles[tt][:, dh * 520:(dh + 1) * 520]
            dest = dest.rearrange("p (h d) -> p h d", d=65)[:, :, 0:64]
            nc.vector.tensor_copy(dest, pt[:])

        def proj_V(wname, lhs_tiles, v_tiles, dh_range=(0, 1)):
            for dh in dh_range:
                wts = []
                for k in range(NT):
                    wt = wpool.tile([128, 512], bf16, tag="wpool", name="vwt")
                    wdma(wt[:], w_d[wname][k * 128:(k + 1) * 128,
                                           dh * 512:(dh + 1) * 512])
                    wts.append(wt)
                for tt in range(NT):
                    pt = pB.tile([128, 512], f32, tag="pB", name="vpt")
                    for k in range(NT):
                        nc.tensor.matmul(pt[:], lhs_tiles[k][:, tt * 128:(tt + 1) * 128],
                                         wts[k][:], start=(k == 0), stop=(k == NT - 1))
                    v_drain(v_tiles, tt, dh, pt)

        # ---------------- filler machinery ----------------
        # Each filler item is a closure emitting ~4 PE matmuls. Groups are
        # split into two chunks (A: k0-3 start, B: k4-7 stop + drain) that
        # must be emitted within the same attention head (pB slot safety).
        fillers = deque()

        def emit_filler(n):
            for _ in range(min(n, len(fillers))):
                fillers.popleft()()

        def drain_fillers():
            while fillers:
                fillers.popleft()()

        def make_v2_fillers(dh):
            # full psum groups (8 matmuls + drain) as self-contained fillers;
            # wcv weight chunks in their own pool, scalar queue (independent
            # of the x/f streams)
            wts = []
            for k in range(NT):
                wt = vwpool.tile([128, 512], bf16, tag="vw", name="vw")
                nc.scalar.dma_start(wt[:], w_d["wcv"][k * 128:(k + 1) * 128,
                                                      dh * 512:(dh + 1) * 512])
                wts.append(wt)
            for tt in range(NT):
                def grp(dh=dh, tt=tt, wts=wts):
                    pt = pB.tile([128, 512], f32, tag="pB", name="v2pt")
                    for k in range(NT):
                        nc.tensor.matmul(pt[:], fb[k][:, tt * 128:(tt + 1) * 128],
                                         wts[k][:], start=(k == 0),
                                         stop=(k == NT - 1))
                    v_drain(v2t, tt, dh, pt)
                fillers.append(grp)

        def make_k2_fillers(mh):
            wts = []
            for k in range(NT):
                wt = vwpool.tile([128, 512], bf16, tag="vw", name="k2w")
                nc.scalar.dma_start(wt[:], w_d["wck"][k * 128:(k + 1) * 128,
                                                      mh * 512:(mh + 1) * 512])
                wts.append(wt)
            for ml in range(4):
                mi = 4 * mh + ml
                for h in range(2):
                    def grp(mi=mi, ml=ml, h=h, wts=wts):
                        pt = pB.tile([128, 512], f32, tag="pB", name="k2pt")
                        for k in range(NT):
                            nc.tensor.matmul(pt[:], wts[k][:, ml * 128:(ml + 1) * 128],
                                             fb[k][:, h * 512:(h + 1) * 512],
                                             start=(k == 0), stop=(k == NT - 1))
                        nc.vector.tensor_copy(k2T[mi][:, h * 512:(h + 1) * 512],
                                              pt[:])
                    fillers.append(grp)

        # ---------------- attention ----------------
        f32r = mybir.dt.float32r

        def attention(q_tiles, k_tiles, v_tiles, o_tiles, pre_pair=None):
            dq = [None]

            def rescale_start():
                # fast recip of 8 denom rows, rounded to f32r for the PE
                dquad = dq[0]
                nc.vector.reciprocal_approx_fast(dquad[:], dquad[:])
                dr = dallp.tile([8, 512], f32r, tag="dallr", name="dallr", bufs=2)
                nc.vector.tensor_copy(dr[:], dquad[:])
                return dr

            def rescale_pair(dr, j, mi2):
                bp = pB.tile([128, 512], f32, tag="pB", name="selbp")
                nc.tensor.matmul(bp[:], sel_t[j][:], dr[:], start=True, stop=True)
                rbc = recb.tile([128, 512], bf16, tag="recb", name="rbc")
                nc.vector.tensor_copy(rbc[:], bp[:])
                nc.vector.tensor_mul(o_tiles[mi2][:], o_tiles[mi2][:], rbc[:])

            dr0 = [None]
            for h in range(H):
                mi, off = h // 2, 64 * (h % 2)
                if h % 8 == 0:
                    dq[0] = dallp.tile([8, 512], f32, tag="dall", name="dquad",
                                       bufs=2)
                if h == 8:
                    # hard deadline: K2-mh1 / V2-dh1 filler groups feed the
                    # scores and AV of heads 8..15 — emit any stragglers now
                    drain_fillers()
                if pre_pair is not None and h % 2 == 0:
                    pre_pair(mi)
                    emit_filler(1)
                if 8 <= h <= 11:
                    # quad-0 pair rescales deferred here so their recip
                    # chain never stalls the PE
                    rescale_pair(dr0[0], h - 8, h - 8)
                op = pB.tile([65, 512], f32, tag="pB", name="op")
                ets = [None] * 4

                def score(jp):
                    kj0, kj1 = 2 * jp, 2 * jp + 1
                    q0, q1 = 64 * kj0, 64 * kj1
                    e1 = 512 + (512 - q1)
                    st = pA.tile([128, 1024], f32, tag="pA", name="st")
                    nc.tensor.matmul(
                        st[:, q0:512],
                        k_tiles[mi][off:off + 64, kj0 * 128:(kj0 + 1) * 128],
                        q_tiles[mi][off:off + 64, q0:512],
                        start=True, stop=True)
                    nc.tensor.matmul(
                        st[:, 512:e1],
                        k_tiles[mi][off:off + 64, kj1 * 128:(kj1 + 1) * 128],
                        q_tiles[mi][off:off + 64, q1:512],
                        start=True, stop=True)
                    et = epool.tile([128, 1024], bf16, tag="epool", name="et")
                    nc.scalar.activation(et[:, q0:e1], st[:, q0:e1], AF.Exp)
                    # multiplicative causal mask on diagonal-straddling blocks
                    nc.vector.tensor_mul(et[:, q0:q0 + 64], et[:, q0:q0 + 64], mask_t[:])
                    nc.vector.tensor_mul(et[:, 512:576], et[:, 512:576], mask_t[:])
                    ets[jp] = et

                def av(jp):
                    kj0, kj1 = 2 * jp, 2 * jp + 1
                    q0, q1 = 64 * kj0, 64 * kj1
                    e1 = 512 + (512 - q1)
                    et = ets[jp]
                    nc.tensor.matmul(
                        op[:] if kj0 == 0 else op[:, q0:512],
                        v_tiles[kj0][:, 65 * h:65 * h + 65],
                        et[:, q0:512], start=(kj0 == 0), stop=False)
                    nc.tensor.matmul(
                        op[:, q1:512],
                        v_tiles[kj1][:, 65 * h:65 * h + 65],
                        et[:, 512:e1], start=False, stop=(kj1 == 7))

                score(0)
                for jp in range(4):
                    if jp == 1:
                        emit_filler(1)
                    if jp < 3:
                        score(jp + 1)
                    av(jp)
                # stash raw head output + denominator row
                nc.vector.tensor_copy(o_tiles[mi][off:off + 64, :], op[0:64, :])
                rr = rrows.tile([1, 512], f32, tag="rr", name="rr")
                nc.vector.tensor_copy(rr[:], op[64:65, :])
                nc.gpsimd.dma_start(dq[0][h % 8:h % 8 + 1, :], rr[:])
                if h == 7:
                    dr0[0] = rescale_start()
                elif h == 15:
                    dr1 = rescale_start()
                    for j in range(4):
                        rescale_pair(dr1, j, 4 + j)

        # =============== phase 1: k2 (mh0) + full-x LN stats ===============
        k2T = [k2pool.tile([128, T], bf16, tag="k2", name=f"k2T{i}") for i in range(NT)]
        xsq = []
        for k in range(NT):
            sq = sqpool.tile([128, T], bf16, tag="sqf", name=f"xsq{k}")
            # Square on the (idle) Scalar engine keeps DVE free for casts
            nc.scalar.activation(sq[:], xb[k][:], AF.Square)
            xsq.append(sq)

        # small constants on the scalar queue, emitted AFTER the Square work
        # so their sequencer dispatch cost never delays the LN stats; the
        # tiles themselves are first read ~25us in (lnb scale/bias)
        mask_t = maskp.tile([128, 64], bf16, tag="mask")
        nc.scalar.dma_start(mask_t[:], mask_d[:, :])
        sel_t = []
        for j in range(4):
            st_ = smalls.tile([8, 128], mybir.dt.float32r, tag=f"sel{j}",
                              name=f"sel{j}")
            nc.scalar.dma_start(st_[:], sel_d[j])
            sel_t.append(st_)

        def load_percol(name, n=NT):
            t = smalls.tile([128, n], f32, tag=name, name=name)
            nc.scalar.dma_start(t[:], bias_d[name].rearrange("(m p) -> p m", p=128))
            return t

        bias_t = {
            n: load_percol(n)
            for n in ["bq", "bsp", "bcq", "bcp", "bpr", "g1", "b1", "g2", "b2"]
        }
        bias_t["bfc"] = load_percol("bfc", 32)

        mean_ps = pA.tile([1, T], f32, tag="pA", name="meanps")
        sq_ps = pA.tile([1, T], f32, tag="pA", name="sqps")
        stat_cnt = [0]

        def emit_stats_upto(n):
            while stat_cnt[0] < n:
                k = stat_cnt[0]
                for hh in range(2):
                    sl = slice(hh * 512, (hh + 1) * 512)
                    nc.tensor.matmul(mean_ps[0:1, sl], ones_c16[:], xb[k][:, sl],
                                     start=(k == 0), stop=(k == NT - 1))
                    nc.tensor.matmul(sq_ps[0:1, sl], ones_c16[:], xsq[k][:, sl],
                                     start=(k == 0), stop=(k == NT - 1))
                stat_cnt[0] += 1

        emit_stats_upto(NT)

        # full-x LN: rows + bcast per 512-half, f32 intermediates
        mb_f = sbig.tile([128, T], f32, tag="sbig", name="mbf")
        rb_f = sbig.tile([128, T], f32, tag="sbig", name="rbf")
        for hh in range(2):
            sl = slice(hh * 512, (hh + 1) * 512)
            mean_row, rstd_row = stat_rows(mean_ps[0:1, sl], sq_ps[0:1, sl], 512)
            bcast_rows_f32(mean_row, mb_f[:, sl], 512)
            bcast_rows_f32(rstd_row, rb_f[:, sl], 512)

        lnb = xb  # bf16 x tiles overwritten with LN output
        for k in range(NT):
            t32 = big32.tile([128, T], f32, tag="big32", name="lnt32f")
            nc.vector.tensor_sub(t32[:], xb[k][:], mb_f[:])
            nc.vector.tensor_mul(t32[:], t32[:], rb_f[:])
            nc.scalar.activation(lnb[k][:], t32[:], AF.Identity,
                                 bias=bias_t["b1"][:, k:k + 1],
                                 scale=bias_t["g1"][:, k:k + 1])

        def dump(name, tiles):
            if debug:
                for k in range(NT):
                    nc.sync.dma_start(dbg[name][k * 128:(k + 1) * 128, :],
                                      tiles[k][:])

        dump("dbg_lnb", lnb)

        # =============== phase 2: k, v projections ===============
        kT = [kpool.tile([128, T], bf16, tag="kT", name=f"kT{i}") for i in range(NT)]

        def k_drain(mi, pt, h):
            if (mi + h) % 2 == 0:
                nc.vector.tensor_copy(kT[mi][:, h * 512:(h + 1) * 512], pt[:])
            else:
                nc.scalar.copy(kT[mi][:, h * 512:(h + 1) * 512], pt[:])

        projT("wk", lnb, T, k_drain)
        dump("dbg_k", kT)
        proj_V("wv", lnb, vt)

        # own-token LN from residual (f32) -> lnown; emitted after wk/wv so
        # its slow input chain (xq stream + busy DVE) never blocks them in
        # PE program order (lnown is first needed by wq inside attention)
        lnown = [lnsm.tile([128, TQ], bf16, tag="lnsm", name=f"lnown{i}")
                 for i in range(NT)]
        ln_small(resid, bias_t["g1"], bias_t["b1"], lnown)
        dump("dbg_lnown", lnown)

        # q-projection groups are emitted inside the attention head loop
        # (pre_pair) so they fill the Scalar-exp stalls
        def make_q_pre(wname, src_tiles, out_tiles, bias_name):
            wts_cur = {}

            def load_half(mh):
                wts = []
                for k in range(NT):
                    wt = wpool.tile([128, 512], bf16, tag="wpool", name="qw")
                    wdma(wt[:], w_d[wname][k * 128:(k + 1) * 128,
                                           mh * 512:(mh + 1) * 512])
                    wts.append(wt)
                wts_cur[0] = wts

            def pre(mi):
                if mi % 4 == 0:
                    load_half(mi // 4)
                ml = mi % 4
                pt = pB.tile([128, 512], f32, tag="pB", name="qpt")
                for k in range(NT):
                    nc.tensor.matmul(pt[:], wts_cur[0][k][:, ml * 128:(ml + 1) * 128],
                                     src_tiles[k][:], start=(k == 0),
                                     stop=(k == NT - 1))
                nc.scalar.activation(out_tiles[mi][:], pt[:], AF.Identity,
                                     bias=bias_t[bias_name][:, mi:mi + 1])
            return pre

        # f stream now (queues deliver after the qkv weights), then the
        # feature-side filler groups become available
        for k in range(NT):
            ft = big32.tile([128, T], f32, tag="big32", name=f"fstream{k}")
            (nc.sync if k % 2 == 0 else nc.gpsimd).dma_start(
                ft[:], fT_d[k * 128:(k + 1) * 128, :])
            fbt = acts.tile([128, T], bf16, tag="acts", name=f"fb{k}")
            nc.vector.tensor_copy(fbt[:], ft[:])
            fb.append(fbt)
        make_k2_fillers(0)
        make_v2_fillers(0)  # dh1 moves to the cross window (it starves)

        # =============== phase 3: self attention (wq + K2/V2 fillers) =======
        qT = [qpool.tile([128, TQ], bf16, tag="q", name=f"qT{i}") for i in range(NT)]
        oT = [opool.tile([128, TQ], bf16, tag="o", name=f"oT{i}") for i in range(NT)]
        attention(qT, kT, vt, oT,
                  pre_pair=make_q_pre("wq", lnown, qT, "bq"))
        drain_fillers()
        dump("dbg_q", qT)
        dump("dbg_o", oT)

        # =============== phase 4: self proj + residual ===============
        def sp_drain(mi, pt, h):
            nc.vector.scalar_tensor_tensor(resid[mi][:], pt[:],
                                           bias_t["bsp"][:, mi:mi + 1],
                                           resid[mi][:], op0=AL.add, op1=AL.add)

        projT("wsp", oT, TQ, sp_drain)
        dump("dbg_r1", resid)

        # =============== phase 5: LN1 on updated own tokens ===============
        make_k2_fillers(1)
        make_v2_fillers(1)
        ln1b = [lnsm.tile([128, TQ], bf16, tag="lnsm", name=f"ln1b{i}")
                for i in range(NT)]
        ln_small(resid, bias_t["g1"], bias_t["b1"], ln1b, emit_filler=emit_filler)

        # =============== phase 6+7: cross attention (wcq + k2 fillers) ======
        q2T = [qpool.tile([128, TQ], bf16, tag="q", name=f"q2T{i}") for i in range(NT)]
        o2T = [opool.tile([128, TQ], bf16, tag="o", name=f"o2T{i}") for i in range(NT)]
        attention(q2T, k2T, v2t, o2T,
                  pre_pair=make_q_pre("wcq", ln1b, q2T, "bcq"))
        drain_fillers()
        dump("dbg_o2", o2T)
        dump("dbg_k2", k2T)

        # =============== phase 8: cross proj + residual ===============
        def cp_drain(mi, pt, h):
            nc.vector.scalar_tensor_tensor(resid[mi][:], pt[:],
                                           bias_t["bcp"][:, mi:mi + 1],
                                           resid[mi][:], op0=AL.add, op1=AL.add)

        projT("wcp", o2T, TQ, cp_drain)
        dump("dbg_r2", resid)

        # =============== phase 9: LN2 + MLP ===============
        def load_fc_w(grp):
            wts = []
            for k in range(NT):
                wt = wpool.tile([128, 512], bf16, tag="wpool", name="fcw")
                wdma(wt[:], w_d["wfc"][k * 128:(k + 1) * 128,
                                       grp * 512:(grp + 1) * 512])
                wts.append(wt)
            return wts

        fc_w0 = load_fc_w(0)  # prefetch while LN2 runs
        ln2 = [lnsm.tile([128, TQ], bf16, tag="lnsm", name=f"ln2_{i}")
               for i in range(NT)]
        ln_small(resid, bias_t["g2"], bias_t["b2"], ln2)

        # hidden tiles reuse dead kT/k2T/v slots (no extra SBUF)
        m_sb = []
        for i in range(32):
            if i < 8:
                mt = kpool.tile([128, TQ], bf16, tag="kT", name=f"m{i}")
            elif i < 16:
                mt = k2pool.tile([128, TQ], bf16, tag="k2", name=f"m{i}")
            else:
                mt = vpool.tile([128, TQ], bf16, tag="vpool", name=f"m{i}")
            m_sb.append(mt)

        # pr accumulators quad0 (mi 0..3) live across fc; fc psums from pA
        pr_ps0 = [pB.tile([128, TQ], f32, tag="pB", name=f"pr0_{j}")
                  for j in range(4)]

        def fc_group(grp, wts):
            for ml in range(4):
                mi = grp * 4 + ml
                pt = pA.tile([128, TQ], f32, tag="pA", name="fcpt")
                for k in range(NT):
                    nc.tensor.matmul(pt[:, 0:TQ], wts[k][:, ml * 128:(ml + 1) * 128],
                                     ln2[k][:], start=(k == 0), stop=(k == NT - 1))
                nc.scalar.activation(m_sb[mi][:], pt[:, 0:TQ], AF.Gelu_apprx_tanh,
                                     bias=bias_t["bfc"][:, mi:mi + 1])

        def pr_q0_group(k):
            wt = wpool.tile([128, 512], bf16, tag="wpool", name="prw")
            wdma(wt[:], w_d["wpr"][k * 128:(k + 1) * 128, 0:512])
            for j in range(4):
                nc.tensor.matmul(pr_ps0[j][:], wt[:, j * 128:(j + 1) * 128],
                                 m_sb[k][:], start=(k == 0), stop=(k == 31))

        # fc groups with lagged pr-quad0 interleave (pr group k after fc
        # group covering hidden tile k is complete)
        pr_done = [0]

        def pump_pr(n):
            while pr_done[0] < n:
                pr_q0_group(pr_done[0])
                pr_done[0] += 1

        fc_wts = {0: fc_w0}
        for grp in range(8):
            wts = fc_wts.pop(grp)
            if grp + 1 < 8:
                fc_wts[grp + 1] = load_fc_w(grp + 1)
            fc_group(grp, wts)
            if grp >= 1:
                pump_pr(4 * grp)   # lag one group behind gelu
        pump_pr(32)

        def emit_out(quad, qts):
            for j in range(4):
                mi = quad * 4 + j
                of = outfp.tile([128, TQ], f32, tag="outf", name="of")
                nc.vector.scalar_tensor_tensor(of[:], qts[j][:],
                                               bias_t["bpr"][:, mi:mi + 1],
                                               resid[mi][:], op0=AL.add, op1=AL.add)
                nc.sync.dma_start(outT_d[mi * 128:(mi + 1) * 128, :], of[:])

        emit_out(0, pr_ps0)

        # pr quad1 (mi 4..7): straight accumulation, all m_sb ready
        pr_ps1 = [pB.tile([128, TQ], f32, tag="pB", name=f"pr1_{j}")
                  for j in range(4)]
        for k in range(32):
            wt = wpool.tile([128, 512], bf16, tag="wpool", name="prw1")
            wdma(wt[:], w_d["wpr"][k * 128:(k + 1) * 128, 512:1024])
            for j in range(4):
                nc.tensor.matmul(pr_ps1[j][:], wt[:, j * 128:(j + 1) * 128],
                                 m_sb[k][:], start=(k == 0), stop=(k == 31))
        emit_out(1, pr_ps1)

    nc.compile()
    return nc


def _get_program():
    global _PROG
    if _PROG is None:
        _PROG = _build_program()
    return _PROG


def _prep_shared(inputs):
    g = {}

    def bf(a):
        return np.ascontiguousarray(np.asarray(a, dtype=np.float32)).astype(BF)

    def f(a):
        return np.ascontiguousarray(np.asarray(a, dtype=np.float32))

    def fold(w, a, lb):
        # effective W^T (in->out layout) with LoRA folded:
        # y = x W^T + (x A^T) B^T s  ->  W_eff^T = W^T + A^T B^T s
        return np.asarray(w, np.float32).T + \
            np.asarray(a, np.float32).T @ np.asarray(lb, np.float32).T * SCALE

    qw, kw, vw = (inputs["sa_qkv_w"][i * C:(i + 1) * C] for i in range(3))
    qb, kb, vb = (inputs["sa_qkv_b"][i * C:(i + 1) * C] for i in range(3))
    qlb, klb, vlb = (inputs["sa_qkv_lb"][i * C:(i + 1) * C] for i in range(3))
    inv = 1.0 / np.sqrt(DH)
    a_sa = inputs["sa_qkv_a"]
    g["wq"] = bf(fold(qw, a_sa, qlb) * inv)
    g["wk"] = bf(fold(kw, a_sa, klb))
    g["wv"] = bf(fold(vw, a_sa, vlb))
    g["bq"] = f(np.asarray(qb) * inv)
    # K bias dropped: adds a per-query constant to all logits (softmax
    # shift-invariant over keys). V bias folded into the next projection.
    g["wsp"] = bf(fold(inputs["sa_proj_w"], inputs["sa_proj_a"], inputs["sa_proj_lb"]))
    g["bsp"] = f(np.asarray(inputs["sa_proj_b"], np.float32) +
                 np.asarray(inputs["sa_proj_w"], np.float32) @ np.asarray(vb, np.float32))

    g["wcq"] = bf(fold(inputs["ca_q_w"], inputs["ca_q_a"], inputs["ca_q_lb"]) * inv)
    g["bcq"] = f(np.asarray(inputs["ca_q_b"]) * inv)

    ckw, cvw = inputs["ca_kv_w"][0:C], inputs["ca_kv_w"][C:2 * C]
    cvb = inputs["ca_kv_b"][C:2 * C]
    cklb, cvlb = inputs["ca_kv_lb"][0:C], inputs["ca_kv_lb"][C:2 * C]
    a_ck = inputs["ca_kv_a"]
    g["wck"] = bf(fold(ckw, a_ck, cklb))
    g["wcv"] = bf(fold(cvw, a_ck, cvlb))

    g["wcp"] = bf(fold(inputs["ca_proj_w"], inputs["ca_proj_a"], inputs["ca_proj_lb"]))
    g["bcp"] = f(np.asarray(inputs["ca_proj_b"], np.float32) +
                 np.asarray(inputs["ca_proj_w"], np.float32) @ np.asarray(cvb, np.float32))

    g["wfc"] = bf(np.asarray(inputs["fc_w"]).T)
    g["bfc"] = f(inputs["fc_b"])
    g["wpr"] = bf(np.asarray(inputs["pr_w"]).T)
    g["bpr"] = f(inputs["pr_b"])
    g["g1"] = f(inputs["ln1_g"])
    g["b1"] = f(inputs["ln1_b"])
    g["g2"] = f(inputs["ln2_g"])
    g["b2"] = f(inputs["ln2_b"])

    sel = np.zeros((4, 8, 128), np.float32)
    for j in range(4):
        sel[j, 2 * j, 0:64] = 1.0
        sel[j, 2 * j + 1, 64:128] = 1.0
    g["sel"] = sel
    return g


def _make_in_maps(inputs):
    inputs = {k: np.asarray(v) for k, v in inputs.items()}
    x, feat = inputs["x"], inputs["feature"]
    B = x.shape[0]
    shared = _prep_shared(inputs)

    masks = []
    for p in range(2):
        jj = np.arange(128).reshape(128, 1)
        ii = np.arange(64).reshape(1, 64)
        live = jj <= 2 * ii + p
        masks.append(np.where(live, 1.0, 0.0).astype(np.float32).astype(BF))

    in_maps = []
    xTs = [np.ascontiguousarray(np.asarray(x[b]).T, dtype=np.float32) for b in range(B)]
    fTs = [np.ascontiguousarray(np.asarray(feat[b]).T, dtype=np.float32) for b in range(B)]
    for core in range(NCORES):
        b, p = core // 2, core % 2
        m = dict(shared)
        m["xT"] = xTs[b]
        m["xqT"] = np.ascontiguousarray(xTs[b][:, p::2])
        m["fT"] = fTs[b]
        m["mask"] = masks[p]
        in_maps.append(m)
    return in_maps, B


def kernel(**inputs):
    from concourse.bass_utils import run_bass_kernel_spmd

    nc = _get_program()
    in_maps, B = _make_in_maps(inputs)
    res = run_bass_kernel_spmd(nc, in_maps, core_ids=list(range(NCORES)))
    out = np.zeros((B, T, C), np.float32)
    for core in range(NCORES):
        b, p = core // 2, core % 2
        out[b, p::2, :] = np.asarray(res.results[core]["outT"], dtype=np.float32).T
    return out


# revision 111
# speedup vs baseline: 1.0443x; 1.0251x over previous
"""Trainium2 Bass kernel for nn_Block_with_lora (dense transformer block).

Sharding: 8 cores = 4 batches x 2 token-parity shards (stride-2 over T).
Each core computes its 512 query tokens end-to-end (no collectives);
K/V projections over all 1024 tokens are computed per-core.

v2: LoRA folded into weights host-side (W+scale*A*B); K-bias dropped
(softmax shift-invariant over keys); V-bias folded into the following
projection's bias; multiplicative post-exp mask (GpSimd) instead of
additive band; single x load with in-place bf16 LN; single-pass MLP
with fc/pr interleave; V2/K2 projections emitted as in-order PE filler
inside the attention phases so the tensor engine never idles (TRN2
p-state ramps to 2.4GHz only after ~3us of continuous PE activity).
"""

import sys

sys.path.insert(0, "/opt/trn_rl_repo")

import numpy as np
import ml_dtypes
from collections import deque
from contextlib import ExitStack

BF = ml_dtypes.bfloat16

C = 1024
H = 16
DH = 64
R = 16
SCALE = 1.0 / R
T = 1024
TQ = 512
NT = 8  # C / 128
EPS = 1e-5
NCORES = 8

_PROG = None


def _build_program(debug=False):
    import concourse.bass as bass
    import concourse.tile as tile
    from concourse import mybir, bacc

    f32 = mybir.dt.float32
    bf16 = mybir.dt.bfloat16
    AF = mybir.ActivationFunctionType
    AL = mybir.AluOpType

    nc = bacc.Bacc("TRN2", target_bir_lowering=False, debug=False)

    def din(name, shape, dt=f32):
        return nc.dram_tensor(name, shape, dt, kind="ExternalInput").ap()

    xT_d = din("xT", [C, T])
    xqT_d = din("xqT", [C, TQ])
    fT_d = din("fT", [C, T])
    mask_d = din("mask", [128, 64], bf16)
    sel_d = din("sel", [4, 8, 128], mybir.dt.float32r)

    w_d = {}
    for n in ["wq", "wk", "wv", "wsp", "wcq", "wck", "wcv", "wcp"]:
        w_d[n] = din(n, [C, C], bf16)
    w_d["wfc"] = din("wfc", [C, 4 * C], bf16)
    w_d["wpr"] = din("wpr", [4 * C, C], bf16)
    bias_d = {
        n: din(n, [C], f32)
        for n in ["bq", "bsp", "bcq", "bcp", "bpr", "g1", "b1", "g2", "b2"]
    }
    bias_d["bfc"] = din("bfc", [4 * C], f32)

    outT_d = nc.dram_tensor("outT", [C, TQ], f32, kind="ExternalOutput").ap()
    dbg = {}
    if debug:
        for n, sh, dt in [("dbg_lnb", [C, T], bf16), ("dbg_lnown", [C, TQ], bf16),
                          ("dbg_q", [C, TQ], bf16), ("dbg_k", [C, T], bf16),
                          ("dbg_o", [C, TQ], bf16), ("dbg_r1", [C, TQ], f32),
                          ("dbg_o2", [C, TQ], bf16), ("dbg_r2", [C, TQ], f32),
                          ("dbg_k2", [C, T], bf16)]:
            dbg[n] = nc.dram_tensor(n, sh, dt, kind="ExternalOutput").ap()

    with tile.TileContext(nc) as tc, ExitStack() as ctx:

        def pool(name, bufs, space=None):
            kw = dict(name=name, bufs=bufs)
            if space:
                kw["space"] = space
            return ctx.enter_context(tc.tile_pool(**kw))

        # ---- SBUF pools ----
        acts = pool("acts", 16)      # [128,1024] bf16: xb->lnb (8) + fb (8)  32KB
        kpool = pool("kpool", 8)     # [128,1024] bf16: kT (later m_sb 0..7)  16KB
        k2pool = pool("k2pool", 8)   # [128,1024] bf16: k2T (later m_sb 8..15) 16KB
        vpool = pool("vpool", 16)    # [128,1040] bf16: vt+v2t (later m_sb 16..31) 32.5KB
        qpool = pool("qpool", 8)     # [128,512] bf16: qT -> q2T               8KB
        lnsm = pool("lnsm", 8)       # [128,512] bf16: own_b->lnown/ln1b/ln2   8KB
        opool = pool("opool", 8)     # [128,512] bf16: oT -> o2T               8KB
        rpool = pool("rpool", 8)     # [128,512] f32: residual (persist)      16KB
        wpool = pool("wpool", 14)    # [128,512] bf16 weight chunks           14KB
        vwpool = pool("vwpool", 8)   # [128,512] bf16 wcv chunks (filler)      8KB
        epool = pool("epool", 2)     # [128,1024] bf16: exp(S)                 4KB
        big32 = pool("big32", 2)     # [128,1024] f32: x/f stream              8KB
        sbig = pool("sbig", 2)       # [128,1024] bf16: mb/rb bcast            4KB
        sqpool = pool("sqpool", 2)   # [128,1024] bf16: squares                4KB
        rows = pool("rows", 2)       # [1,1024] f32 stat rows                  8KB
        rrows = pool("rrows", 1)     # [1,512] f32 softmax denom rows          2KB
        recb = pool("recb", 1)       # [128,512] bf16 recip bcast              1KB
        dallp = pool("dallp", 1)     # [16,512] f32 batched denoms             2KB
        outfp = pool("outfp", 1)     # [128,512] f32 out staging               2KB
        smalls = pool("smalls", 1)   # bias/g/b columns per tag
        onesp = pool("onesp", 1)
        maskp = pool("maskp", 1)

        # ---- PSUM pools: 2*2 + 4*1 = 8 banks ----
        pA = pool("pA", 2, space="PSUM")   # [128,1024] f32 (2 banks each)
        pB = pool("pB", 4, space="PSUM")   # [128,512] f32 (1 bank each)

        # ---- constants ----
        ones_c16 = onesp.tile([128, 1], bf16, tag="oc16")
        nc.gpsimd.memset(ones_c16[:], 1.0)
        ones_r32 = onesp.tile([1, 128], f32, tag="or32")
        nc.gpsimd.memset(ones_r32[:], 1.0)
        eps_t = onesp.tile([1, 1], f32, tag="eps")
        nc.gpsimd.memset(eps_t[:], EPS)

        # weight DMA round-robin over (gpsimd, sync) queues
        dma_rr = [0]

        def wdma(dst, src):
            eng = (nc.gpsimd, nc.sync)[dma_rr[0] % 2]
            dma_rr[0] += 1
            eng.dma_start(dst, src)

        # =============== input streams ===============
        # x first (it heads the longest serial chain: stats -> LN -> k/v
        # projections), then xq, then f (emitted later, after the qkv
        # weight DMAs, so queues deliver in need-order)
        xb = []
        for k in range(NT):
            xt = big32.tile([128, T], f32, tag="big32", name=f"xstream{k}")
            (nc.sync if k % 2 == 1 else nc.gpsimd).dma_start(
                xt[:], xT_d[k * 128:(k + 1) * 128, :])
            xbt = acts.tile([128, T], bf16, tag="acts", name=f"xb{k}")
            nc.vector.tensor_copy(xbt[:], xt[:])
            xb.append(xbt)
        resid = []
        for k in range(NT):
            rt = rpool.tile([128, TQ], f32, tag="rpool", name=f"resid{k}")
            (nc.sync if k % 2 == 0 else nc.gpsimd).dma_start(
                rt[:], xqT_d[k * 128:(k + 1) * 128, :])
            resid.append(rt)
        fb = []  # filled by load_f() below, after qkv weight DMAs
        # ones columns for V denominators
        vt = [vpool.tile([128, 1040], bf16, tag="vpool", name=f"vt{i}") for i in range(NT)]
        v2t = [vpool.tile([128, 1040], bf16, tag="vpool", name=f"v2t{i}") for i in range(NT)]
        for tt in range(NT):
            nc.gpsimd.memset(vt[tt][:, 64:1040:65], 1.0)
            nc.gpsimd.memset(v2t[tt][:, 64:1040:65], 1.0)

        # =============== helpers ===============
        def bcast_rows_f32(row_f32, out_sb, Tn):
            # broadcast [1,Tn] f32 row to [128,Tn] f32 SBUF via K=1 PE matmul
            for hh in range(Tn // 512):
                sl = slice(hh * 512, (hh + 1) * 512)
                bp = pB.tile([128, 512], f32, tag="pB", name="bcp")
                nc.tensor.matmul(bp[:], ones_r32[0:1, :], row_f32[0:1, sl],
                                 start=True, stop=True)
                nc.vector.tensor_copy(out_sb[:, sl], bp[:])

        def stat_rows(mean_ps, sq_ps, Tn):
            """mean/rstd rows from accumulated sum / sumsq psums (Tn<=512)."""
            mean_row = rows.tile([1, Tn], f32, tag="rows", name="meanr")
            rstd_row = rows.tile([1, Tn], f32, tag="rows", name="rstdr")
            nc.vector.tensor_scalar_mul(mean_row[:], mean_ps[:], 1.0 / C)
            nc.vector.tensor_mul(rstd_row[:], mean_row[:], mean_row[:])
            nc.vector.scalar_tensor_tensor(rstd_row[:], sq_ps[:], 1.0 / C, rstd_row[:],
                                           op0=AL.mult, op1=AL.subtract)
            nc.scalar.activation(rstd_row[:], rstd_row[:], AF.Sqrt, bias=eps_t[:])
            # approx recip (~18 bits): the exact DVE reciprocal is a fixed
            # ~3.5us op sitting in every LN boundary chain
            nc.vector.reciprocal_approx_fast(rstd_row[:], rstd_row[:])
            return mean_row, rstd_row

        def ln_small(src_f32_tiles, g_col, b_col, out_tiles, emit_filler=None):
            """LN over channel (partition) dim for [128,TQ] f32 tiles.
            f32 intermediates throughout (score precision is exp-amplified)."""
            mean_ps = pA.tile([1, TQ], f32, tag="pA", name="mps")
            sq_ps = pA.tile([1, TQ], f32, tag="pA", name="sps")
            for k in range(NT):
                nc.vector.tensor_copy(out_tiles[k][:], src_f32_tiles[k][:])
                sq = sqpool.tile([128, TQ], bf16, tag="sqf", name="sqo")
                nc.scalar.activation(sq[:], out_tiles[k][:], AF.Square)
                nc.tensor.matmul(mean_ps[:], ones_c16[:], out_tiles[k][:],
                                 start=(k == 0), stop=(k == NT - 1))
                nc.tensor.matmul(sq_ps[:], ones_c16[:], sq[:],
                                 start=(k == 0), stop=(k == NT - 1))
            if emit_filler is not None:
                emit_filler(1)
            mean_row, rstd_row = stat_rows(mean_ps, sq_ps, TQ)
            mb = sbig.tile([128, TQ], f32, tag="sbig", name="mbs")
            rb = sbig.tile([128, TQ], f32, tag="sbig", name="rbs")
            bcast_rows_f32(mean_row, mb, TQ)
            bcast_rows_f32(rstd_row, rb, TQ)
            if emit_filler is not None:
                emit_filler(1)
            for k in range(NT):
                t32 = big32.tile([128, TQ], f32, tag="big32", name="lnt32")
                nc.vector.tensor_sub(t32[:], src_f32_tiles[k][:], mb[:])
                nc.vector.tensor_mul(t32[:], t32[:], rb[:])
                nc.scalar.activation(out_tiles[k][:], t32[:], AF.Identity,
                                     bias=b_col[:, k:k + 1], scale=g_col[:, k:k + 1])

        def projT(wname, rhs_tiles, Tn, drain, mh_range=(0, 1), interleave=None,
                  wqueue=None):
            """outT[mi] psum groups; drain(mi, h, pt) consumes each."""
            for mh in mh_range:
                wts = []
                for k in range(NT):
                    wt = wpool.tile([128, 512], bf16, tag="wpool", name="wt")
                    if wqueue is None:
                        wdma(wt[:], w_d[wname][k * 128:(k + 1) * 128,
                                               mh * 512:(mh + 1) * 512])
                    else:
                        wqueue.dma_start(wt[:], w_d[wname][k * 128:(k + 1) * 128,
                                                           mh * 512:(mh + 1) * 512])
                    wts.append(wt)
                for ml in range(4):
                    mi = mh * 4 + ml
                    for h in range(Tn // 512):
                        pt = pB.tile([128, 512], f32, tag="pB", name="pt")
                        for k in range(NT):
                            nc.tensor.matmul(pt[:], wts[k][:, ml * 128:(ml + 1) * 128],
                                             rhs_tiles[k][:, h * 512:(h + 1) * 512],
                                             start=(k == 0), stop=(k == NT - 1))
                        drain(mi, pt, h)
                        if interleave is not None:
                            interleave(mh, ml, h)

        def v_drain(v_tiles, tt, dh, pt):
            dest = v_tiles[tt][:, dh * 520:(dh + 1) * 520]
            dest = dest.rearrange("p (h d) -> p h d", d=65)[:, :, 0:64]
            nc.vector.tensor_copy(dest, pt[:])

        def proj_V(wname, lhs_tiles, v_tiles, dh_range=(0, 1)):
            for dh in dh_range:
                wts = []
                for k in range(NT):
                    wt = wpool.tile([128, 512], bf16, tag="wpool", name="vwt")
                    wdma(wt[:], w_d[wname][k * 128:(k + 1) * 128,
                                           dh * 512:(dh + 1) * 512])
                    wts.append(wt)
                for tt in range(NT):
                    pt = pB.tile([128, 512], f32, tag="pB", name="vpt")
                    for k in range(NT):
                        nc.tensor.matmul(pt[:], lhs_tiles[k][:, tt * 128:(tt + 1) * 128],
                                         wts[k][:], start=(k == 0), stop=(k == NT - 1))
                    v_drain(v_tiles, tt, dh, pt)

        # ---------------- filler machinery ----------------
        # Each filler item is a closure emitting ~4 PE matmuls. Groups are
        # split into two chunks (A: k0-3 start, B: k4-7 stop + drain) that
        # must be emitted within the same attention head (pB slot safety).
        fillers = deque()

        def emit_filler(n):
            for _ in range(min(n, len(fillers))):
                fillers.popleft()()

        def drain_fillers():
            while fillers:
                fillers.popleft()()

        def make_v2_fillers(dh, wqueue):
            # full psum groups (8 matmuls + drain) as self-contained fillers;
            # wcv weight chunks in their own pool on the given queue
            wts = []
            for k in range(NT):
                wt = vwpool.tile([128, 512], bf16, tag="vw", name="vw")
                wqueue.dma_start(wt[:], w_d["wcv"][k * 128:(k + 1) * 128,
                                                   dh * 512:(dh + 1) * 512])
                wts.append(wt)
            for tt in range(NT):
                def grp(dh=dh, tt=tt, wts=wts):
                    pt = pB.tile([128, 512], f32, tag="pB", name="v2pt")
                    for k in range(NT):
                        nc.tensor.matmul(pt[:], fb[k][:, tt * 128:(tt + 1) * 128],
                                         wts[k][:], start=(k == 0),
                                         stop=(k == NT - 1))
                    v_drain(v2t, tt, dh, pt)
                fillers.append(grp)

        def make_k2_fillers(mh, wqueue):
            wts = []
            for k in range(NT):
                wt = vwpool.tile([128, 512], bf16, tag="vw", name="k2w")
                wqueue.dma_start(wt[:], w_d["wck"][k * 128:(k + 1) * 128,
                                                   mh * 512:(mh + 1) * 512])
                wts.append(wt)
            for ml in range(4):
                mi = 4 * mh + ml
                for h in range(2):
                    def grp(mi=mi, ml=ml, h=h, wts=wts):
                        pt = pB.tile([128, 512], f32, tag="pB", name="k2pt")
                        for k in range(NT):
                            nc.tensor.matmul(pt[:], wts[k][:, ml * 128:(ml + 1) * 128],
                                             fb[k][:, h * 512:(h + 1) * 512],
                                             start=(k == 0), stop=(k == NT - 1))
                        nc.vector.tensor_copy(k2T[mi][:, h * 512:(h + 1) * 512],
                                              pt[:])
                    fillers.append(grp)

        # ---------------- attention ----------------
        f32r = mybir.dt.float32r

        def attention(q_tiles, k_tiles, v_tiles, o_tiles, pre_pair=None):
            dq = [None]

            def rescale_start():
                # fast recip of 8 denom rows, rounded to f32r for the PE
                dquad = dq[0]
                nc.vector.reciprocal_approx_fast(dquad[:], dquad[:])
                dr = dallp.tile([8, 512], f32r, tag="dallr", name="dallr", bufs=2)
                nc.vector.tensor_copy(dr[:], dquad[:])
                return dr

            def rescale_pair(dr, j, mi2):
                bp = pB.tile([128, 512], f32, tag="pB", name="selbp")
                nc.tensor.matmul(bp[:], sel_t[j][:], dr[:], start=True, stop=True)
                rbc = recb.tile([128, 512], bf16, tag="recb", name="rbc")
                nc.vector.tensor_copy(rbc[:], bp[:])
                nc.vector.tensor_mul(o_tiles[mi2][:], o_tiles[mi2][:], rbc[:])

            dr0 = [None]
            for h in range(H):
                mi, off = h // 2, 64 * (h % 2)
                if h % 8 == 0:
                    dq[0] = dallp.tile([8, 512], f32, tag="dall", name="dquad",
                                       bufs=2)
                if h == 8:
                    # hard deadline: K2-mh1 / V2-dh1 filler groups feed the
                    # scores and AV of heads 8..15 — emit any stragglers now
                    drain_fillers()
                if pre_pair is not None and h % 2 == 0:
                    pre_pair(mi)
                    emit_filler(1)
                if 8 <= h <= 11:
                    # quad-0 pair rescales deferred here so their recip
                    # chain never stalls the PE
                    rescale_pair(dr0[0], h - 8, h - 8)
                op = pB.tile([65, 512], f32, tag="pB", name="op")
                ets = [None] * 4

                def score(jp):
                    kj0, kj1 = 2 * jp, 2 * jp + 1
                    q0, q1 = 64 * kj0, 64 * kj1
                    e1 = 512 + (512 - q1)
                    st = pA.tile([128, 1024], f32, tag="pA", name="st")
                    nc.tensor.matmul(
                        st[:, q0:512],
                        k_tiles[mi][off:off + 64, kj0 * 128:(kj0 + 1) * 128],
                        q_tiles[mi][off:off + 64, q0:512],
                        start=True, stop=True)
                    nc.tensor.matmul(
                        st[:, 512:e1],
                        k_tiles[mi][off:off + 64, kj1 * 128:(kj1 + 1) * 128],
                        q_tiles[mi][off:off + 64, q1:512],
                        start=True, stop=True)
                    et = epool.tile([128, 1024], bf16, tag="epool", name="et")
                    nc.scalar.activation(et[:, q0:e1], st[:, q0:e1], AF.Exp)
                    # multiplicative causal mask on diagonal-straddling blocks
                    nc.vector.tensor_mul(et[:, q0:q0 + 64], et[:, q0:q0 + 64], mask_t[:])
                    nc.vector.tensor_mul(et[:, 512:576], et[:, 512:576], mask_t[:])
                    ets[jp] = et

                def av(jp):
                    kj0, kj1 = 2 * jp, 2 * jp + 1
                    q0, q1 = 64 * kj0, 64 * kj1
                    e1 = 512 + (512 - q1)
                    et = ets[jp]
                    nc.tensor.matmul(
                        op[:] if kj0 == 0 else op[:, q0:512],
                        v_tiles[kj0][:, 65 * h:65 * h + 65],
                        et[:, q0:512], start=(kj0 == 0), stop=False)
                    nc.tensor.matmul(
                        op[:, q1:512],
                        v_tiles[kj1][:, 65 * h:65 * h + 65],
                        et[:, 512:e1], start=False, stop=(kj1 == 7))

                score(0)
                for jp in range(4):
                    if jp == 1:
                        emit_filler(1)
                    if jp < 3:
                        score(jp + 1)
                    av(jp)
                # stash raw head output + denominator row
                nc.vector.tensor_copy(o_tiles[mi][off:off + 64, :], op[0:64, :])
                rr = rrows.tile([1, 512], f32, tag="rr", name="rr")
                nc.vector.tensor_copy(rr[:], op[64:65, :])
                nc.gpsimd.dma_start(dq[0][h % 8:h % 8 + 1, :], rr[:])
                if h == 7:
                    dr0[0] = rescale_start()
                elif h == 15:
                    dr1 = rescale_start()
                    for j in range(4):
                        rescale_pair(dr1, j, 4 + j)

        # =============== phase 1: k2 (mh0) + full-x LN stats ===============
        k2T = [k2pool.tile([128, T], bf16, tag="k2", name=f"k2T{i}") for i in range(NT)]
        xsq = []
        for k in range(NT):
            sq = sqpool.tile([128, T], bf16, tag="sqf", name=f"xsq{k}")
            # Square on the (idle) Scalar engine keeps DVE free for casts
            nc.scalar.activation(sq[:], xb[k][:], AF.Square)
            xsq.append(sq)

        # small constants on the scalar queue, emitted AFTER the Square work
        # so their sequencer dispatch cost never delays the LN stats; the
        # tiles themselves are first read ~25us in (lnb scale/bias)
        mask_t = maskp.tile([128, 64], bf16, tag="mask")
        nc.scalar.dma_start(mask_t[:], mask_d[:, :])
        sel_t = []
        for j in range(4):
            st_ = smalls.tile([8, 128], mybir.dt.float32r, tag=f"sel{j}",
                              name=f"sel{j}")
            nc.scalar.dma_start(st_[:], sel_d[j])
            sel_t.append(st_)

        def load_percol(name, n=NT):
            t = smalls.tile([128, n], f32, tag=name, name=name)
            nc.scalar.dma_start(t[:], bias_d[name].rearrange("(m p) -> p m", p=128))
            return t

        bias_t = {
            n: load_percol(n)
            for n in ["bq", "bsp", "bcq", "bcp", "bpr", "g1", "b1", "g2", "b2"]
        }
        bias_t["bfc"] = load_percol("bfc", 32)

        mean_ps = pA.tile([1, T], f32, tag="pA", name="meanps")
        sq_ps = pA.tile([1, T], f32, tag="pA", name="sqps")
        stat_cnt = [0]

        def emit_stats_upto(n):
            while stat_cnt[0] < n:
                k = stat_cnt[0]
                for hh in range(2):
                    sl = slice(hh * 512, (hh + 1) * 512)
                    nc.tensor.matmul(mean_ps[0:1, sl], ones_c16[:], xb[k][:, sl],
                                     start=(k == 0), stop=(k == NT - 1))
                    nc.tensor.matmul(sq_ps[0:1, sl], ones_c16[:], xsq[k][:, sl],
                                     start=(k == 0), stop=(k == NT - 1))
                stat_cnt[0] += 1

        emit_stats_upto(NT)

        # full-x LN: rows + bcast per 512-half, f32 intermediates
        mb_f = sbig.tile([128, T], f32, tag="sbig", name="mbf")
        rb_f = sbig.tile([128, T], f32, tag="sbig", name="rbf")
        for hh in range(2):
            sl = slice(hh * 512, (hh + 1) * 512)
            mean_row, rstd_row = stat_rows(mean_ps[0:1, sl], sq_ps[0:1, sl], 512)
            bcast_rows_f32(mean_row, mb_f[:, sl], 512)
            bcast_rows_f32(rstd_row, rb_f[:, sl], 512)

        lnb = xb  # bf16 x tiles overwritten with LN output
        for k in range(NT):
            t32 = big32.tile([128, T], f32, tag="big32", name="lnt32f")
            nc.vector.tensor_sub(t32[:], xb[k][:], mb_f[:])
            nc.vector.tensor_mul(t32[:], t32[:], rb_f[:])
            nc.scalar.activation(lnb[k][:], t32[:], AF.Identity,
                                 bias=bias_t["b1"][:, k:k + 1],
                                 scale=bias_t["g1"][:, k:k + 1])

        def dump(name, tiles):
            if debug:
                for k in range(NT):
                    nc.sync.dma_start(dbg[name][k * 128:(k + 1) * 128, :],
                                      tiles[k][:])

        dump("dbg_lnb", lnb)

        # =============== phase 2: k, v projections ===============
        kT = [kpool.tile([128, T], bf16, tag="kT", name=f"kT{i}") for i in range(NT)]

        def k_drain(mi, pt, h):
            if (mi + h) % 2 == 0:
                nc.vector.tensor_copy(kT[mi][:, h * 512:(h + 1) * 512], pt[:])
            else:
                nc.scalar.copy(kT[mi][:, h * 512:(h + 1) * 512], pt[:])

        projT("wk", lnb, T, k_drain)
        dump("dbg_k", kT)
        proj_V("wv", lnb, vt)

        # own-token LN from residual (f32) -> lnown; emitted after wk/wv so
        # its slow input chain (xq stream + busy DVE) never blocks them in
        # PE program order (lnown is first needed by wq inside attention)
        lnown = [lnsm.tile([128, TQ], bf16, tag="lnsm", name=f"lnown{i}")
                 for i in range(NT)]
        ln_small(resid, bias_t["g1"], bias_t["b1"], lnown)
        dump("dbg_lnown", lnown)

        # q-projection groups are emitted inside the attention head loop
        # (pre_pair) so they fill the Scalar-exp stalls
        def make_q_pre(wname, src_tiles, out_tiles, bias_name):
            wts_cur = {}

            def load_half(mh):
                wts = []
                for k in range(NT):
                    wt = wpool.tile([128, 512], bf16, tag="wpool", name="qw")
                    wdma(wt[:], w_d[wname][k * 128:(k + 1) * 128,
                                           mh * 512:(mh + 1) * 512])
                    wts.append(wt)
                wts_cur[0] = wts

            def pre(mi):
                if mi % 4 == 0:
                    load_half(mi // 4)
                ml = mi % 4
                pt = pB.tile([128, 512], f32, tag="pB", name="qpt")
                for k in range(NT):
                    nc.tensor.matmul(pt[:], wts_cur[0][k][:, ml * 128:(ml + 1) * 128],
                                     src_tiles[k][:], start=(k == 0),
                                     stop=(k == NT - 1))
                nc.scalar.activation(out_tiles[mi][:], pt[:], AF.Identity,
                                     bias=bias_t[bias_name][:, mi:mi + 1])
            return pre

        # f stream now (queues deliver after the qkv weights), then the
        # feature-side filler groups become available
        for k in range(NT):
            ft = big32.tile([128, T], f32, tag="big32", name=f"fstream{k}")
            (nc.sync if k % 2 == 0 else nc.gpsimd).dma_start(
                ft[:], fT_d[k * 128:(k + 1) * 128, :])
            fbt = acts.tile([128, T], bf16, tag="acts", name=f"fb{k}")
            nc.vector.tensor_copy(fbt[:], ft[:])
            fb.append(fbt)
        make_k2_fillers(0, nc.scalar)
        make_v2_fillers(0, nc.scalar)  # dh1 moves to the cross window

        # =============== phase 3: self attention (wq + K2/V2 fillers) =======
        qT = [qpool.tile([128, TQ], bf16, tag="q", name=f"qT{i}") for i in range(NT)]
        oT = [opool.tile([128, TQ], bf16, tag="o", name=f"oT{i}") for i in range(NT)]
        attention(qT, kT, vt, oT,
                  pre_pair=make_q_pre("wq", lnown, qT, "bq"))
        drain_fillers()
        dump("dbg_q", qT)
        dump("dbg_o", oT)

        # =============== phase 4: self proj + residual ===============
        def sp_drain(mi, pt, h):
            nc.vector.scalar_tensor_tensor(resid[mi][:], pt[:],
                                           bias_t["bsp"][:, mi:mi + 1],
                                           resid[mi][:], op0=AL.add, op1=AL.add)

        projT("wsp", oT, TQ, sp_drain)
        dump("dbg_r1", resid)

        # =============== phase 5: LN1 on updated own tokens ===============
        # gpsimd queue is empty by now and its dispatch is ~25ns (vs 667ns
        # on the scalar sequencer, which is busy with ln1b activations)
        make_k2_fillers(1, nc.gpsimd)
        make_v2_fillers(1, nc.gpsimd)
        ln1b = [lnsm.tile([128, TQ], bf16, tag="lnsm", name=f"ln1b{i}")
                for i in range(NT)]
        ln_small(resid, bias_t["g1"], bias_t["b1"], ln1b, emit_filler=emit_filler)

        # =============== phase 6+7: cross attention (wcq + k2 fillers) ======
        q2T = [qpool.tile([128, TQ], bf16, tag="q", name=f"q2T{i}") for i in range(NT)]
        o2T = [opool.tile([128, TQ], bf16, tag="o", name=f"o2T{i}") for i in range(NT)]
        attention(q2T, k2T, v2t, o2T,
                  pre_pair=make_q_pre("wcq", ln1b, q2T, "bcq"))
        drain_fillers()
        dump("dbg_o2", o2T)
        dump("dbg_k2", k2T)

        # =============== phase 8: cross proj + residual ===============
        def cp_drain(mi, pt, h):
            nc.vector.scalar_tensor_tensor(resid[mi][:], pt[:],
                                           bias_t["bcp"][:, mi:mi + 1],
                                           resid[mi][:], op0=AL.add, op1=AL.add)

        projT("wcp", o2T, TQ, cp_drain)
        dump("dbg_r2", resid)

        # =============== phase 9: LN2 + MLP ===============
        def load_fc_w(grp):
            wts = []
            for k in range(NT):
                wt = wpool.tile([128, 512], bf16, tag="wpool", name="fcw")
                wdma(wt[:], w_d["wfc"][k * 128:(k + 1) * 128,
                                       grp * 512:(grp + 1) * 512])
                wts.append(wt)
            return wts

        fc_w0 = load_fc_w(0)  # prefetch while LN2 runs
        ln2 = [lnsm.tile([128, TQ], bf16, tag="lnsm", name=f"ln2_{i}")
               for i in range(NT)]
        ln_small(resid, bias_t["g2"], bias_t["b2"], ln2)

        # hidden tiles reuse dead kT/k2T/v slots (no extra SBUF)
        m_sb = []
        for i in range(32):
            if i < 8:
                mt = kpool.tile([128, TQ], bf16, tag="kT", name=f"m{i}")
            elif i < 16:
                mt = k2pool.tile([128, TQ], bf16, tag="k2", name=f"m{i}")
            else:
                mt = vpool.tile([128, TQ], bf16, tag="vpool", name=f"m{i}")
            m_sb.append(mt)

        # pr accumulators quad0 (mi 0..3) live across fc; fc psums from pA
        pr_ps0 = [pB.tile([128, TQ], f32, tag="pB", name=f"pr0_{j}")
                  for j in range(4)]

        def fc_group(grp, wts):
            for ml in range(4):
                mi = grp * 4 + ml
                pt = pA.tile([128, TQ], f32, tag="pA", name="fcpt")
                for k in range(NT):
                    nc.tensor.matmul(pt[:, 0:TQ], wts[k][:, ml * 128:(ml + 1) * 128],
                                     ln2[k][:], start=(k == 0), stop=(k == NT - 1))
                nc.scalar.activation(m_sb[mi][:], pt[:, 0:TQ], AF.Gelu_apprx_tanh,
                                     bias=bias_t["bfc"][:, mi:mi + 1])

        def pr_q0_group(k):
            wt = wpool.tile([128, 512], bf16, tag="wpool", name="prw")
            wdma(wt[:], w_d["wpr"][k * 128:(k + 1) * 128, 0:512])
            for j in range(4):
                nc.tensor.matmul(pr_ps0[j][:], wt[:, j * 128:(j + 1) * 128],
                                 m_sb[k][:], start=(k == 0), stop=(k == 31))

        # fc groups with lagged pr-quad0 interleave (pr group k after fc
        # group covering hidden tile k is complete)
        pr_done = [0]

        def pump_pr(n):
            while pr_done[0] < n:
                pr_q0_group(pr_done[0])
                pr_done[0] += 1

        fc_wts = {0: fc_w0}
        for grp in range(8):
            wts = fc_wts.pop(grp)
            if grp + 1 < 8:
                fc_wts[grp + 1] = load_fc_w(grp + 1)
            fc_group(grp, wts)
            if grp >= 1:
                pump_pr(4 * grp)   # lag one group behind gelu
        pump_pr(32)

        def emit_out(quad, qts):
            for j in range(4):
                mi = quad * 4 + j
                of = outfp.tile([128, TQ], f32, tag="outf", name="of")
                nc.vector.scalar_tensor_tensor(of[:], qts[j][:],
                                               bias_t["bpr"][:, mi:mi + 1],
                                               resid[mi][:], op0=AL.add, op1=AL.add)
                nc.sync.dma_start(outT_d[mi * 128:(mi + 1) * 128, :], of[:])

        emit_out(0, pr_ps0)

        # pr quad1 (mi 4..7): straight accumulation, all m_sb ready
        pr_ps1 = [pB.tile([128, TQ], f32, tag="pB", name=f"pr1_{j}")
                  for j in range(4)]
        for k in range(32):
            wt = wpool.tile([128, 512], bf16, tag="wpool", name="prw1")
            wdma(wt[:], w_d["wpr"][k * 128:(k + 1) * 128, 512:1024])
            for j in range(4):
                nc.tensor.matmul(pr_ps1[j][:], wt[:, j * 128:(j + 1) * 128],
                                 m_sb[k][:], start=(k == 0), stop=(k == 31))
        emit_out(1, pr_ps1)

    nc.compile()
    return nc


def _get_program():
    global _PROG
    if _PROG is None:
        _PROG = _build_program()
    return _PROG


def _prep_shared(inputs):
    g = {}

    def bf(a):
        return np.ascontiguousarray(np.asarray(a, dtype=np.float32)).astype(BF)

    def f(a):
        return np.ascontiguousarray(np.asarray(a, dtype=np.float32))

    def fold(w, a, lb):
        # effective W^T (in->out layout) with LoRA folded:
        # y = x W^T + (x A^T) B^T s  ->  W_eff^T = W^T + A^T B^T s
        return np.asarray(w, np.float32).T + \
            np.asarray(a, np.float32).T @ np.asarray(lb, np.float32).T * SCALE

    qw, kw, vw = (inputs["sa_qkv_w"][i * C:(i + 1) * C] for i in range(3))
    qb, kb, vb = (inputs["sa_qkv_b"][i * C:(i + 1) * C] for i in range(3))
    qlb, klb, vlb = (inputs["sa_qkv_lb"][i * C:(i + 1) * C] for i in range(3))
    inv = 1.0 / np.sqrt(DH)
    a_sa = inputs["sa_qkv_a"]
    g["wq"] = bf(fold(qw, a_sa, qlb) * inv)
    g["wk"] = bf(fold(kw, a_sa, klb))
    g["wv"] = bf(fold(vw, a_sa, vlb))
    g["bq"] = f(np.asarray(qb) * inv)
    # K bias dropped: adds a per-query constant to all logits (softmax
    # shift-invariant over keys). V bias folded into the next projection.
    g["wsp"] = bf(fold(inputs["sa_proj_w"], inputs["sa_proj_a"], inputs["sa_proj_lb"]))
    g["bsp"] = f(np.asarray(inputs["sa_proj_b"], np.float32) +
                 np.asarray(inputs["sa_proj_w"], np.float32) @ np.asarray(vb, np.float32))

    g["wcq"] = bf(fold(inputs["ca_q_w"], inputs["ca_q_a"], inputs["ca_q_lb"]) * inv)
    g["bcq"] = f(np.asarray(inputs["ca_q_b"]) * inv)

    ckw, cvw = inputs["ca_kv_w"][0:C], inputs["ca_kv_w"][C:2 * C]
    cvb = inputs["ca_kv_b"][C:2 * C]
    cklb, cvlb = inputs["ca_kv_lb"][0:C], inputs["ca_kv_lb"][C:2 * C]
    a_ck = inputs["ca_kv_a"]
    g["wck"] = bf(fold(ckw, a_ck, cklb))
    g["wcv"] = bf(fold(cvw, a_ck, cvlb))

    g["wcp"] = bf(fold(inputs["ca_proj_w"], inputs["ca_proj_a"], inputs["ca_proj_lb"]))
    g["bcp"] = f(np.asarray(inputs["ca_proj_b"], np.float32) +
                 np.asarray(inputs["ca_proj_w"], np.float32) @ np.asarray(cvb, np.float32))

    g["wfc"] = bf(np.asarray(inputs["fc_w"]).T)
    g["bfc"] = f(inputs["fc_b"])
    g["wpr"] = bf(np.asarray(inputs["pr_w"]).T)
    g["bpr"] = f(inputs["pr_b"])
    g["g1"] = f(inputs["ln1_g"])
    g["b1"] = f(inputs["ln1_b"])
    g["g2"] = f(inputs["ln2_g"])
    g["b2"] = f(inputs["ln2_b"])

    sel = np.zeros((4, 8, 128), np.float32)
    for j in range(4):
        sel[j, 2 * j, 0:64] = 1.0
        sel[j, 2 * j + 1, 64:128] = 1.0
    g["sel"] = sel
    return g


def _make_in_maps(inputs):
    inputs = {k: np.asarray(v) for k, v in inputs.items()}
    x, feat = inputs["x"], inputs["feature"]
    B = x.shape[0]
    shared = _prep_shared(inputs)

    masks = []
    for p in range(2):
        jj = np.arange(128).reshape(128, 1)
        ii = np.arange(64).reshape(1, 64)
        live = jj <= 2 * ii + p
        masks.append(np.where(live, 1.0, 0.0).astype(np.float32).astype(BF))

    in_maps = []
    xTs = [np.ascontiguousarray(np.asarray(x[b]).T, dtype=np.float32) for b in range(B)]
    fTs = [np.ascontiguousarray(np.asarray(feat[b]).T, dtype=np.float32) for b in range(B)]
    for core in range(NCORES):
        b, p = core // 2, core % 2
        m = dict(shared)
        m["xT"] = xTs[b]
        m["xqT"] = np.ascontiguousarray(xTs[b][:, p::2])
        m["fT"] = fTs[b]
        m["mask"] = masks[p]
        in_maps.append(m)
    return in_maps, B


def kernel(**inputs):
    from concourse.bass_utils import run_bass_kernel_spmd

    nc = _get_program()
    in_maps, B = _make_in_maps(inputs)
    res = run_bass_kernel_spmd(nc, in_maps, core_ids=list(range(NCORES)))
    out = np.zeros((B, T, C), np.float32)
    for core in range(NCORES):
        b, p = core // 2, core % 2
        out[b, p::2, :] = np.asarray(res.results[core]["outT"], dtype=np.float32).T
    return out
